# revision 7
# baseline (speedup 1.0000x reference)
import sys

sys.path.insert(0, "/opt/trn_rl_repo")

import numpy as np
import ml_dtypes

import concourse.bass as bass
import concourse.mybir as mybir
import concourse.tile as tile
from concourse import bacc
from concourse.bass_utils import run_bass_kernel_spmd

BF16 = ml_dtypes.bfloat16
FP8 = ml_dtypes.float8_e4m3
F32 = mybir.dt.float32
BF = mybir.dt.bfloat16
F8 = mybir.dt.float8e4
ALU = mybir.AluOpType
ACTF = mybir.ActivationFunctionType
AX = mybir.AxisListType
PM = mybir.MatmulPerfMode

NCORES = 8
B = 256
BL = B // NCORES          # 32 local batch
REC = 102400
RECL = REC // NCORES      # 12800 local output cols
NW = RECL // 512          # 25 output windows

S_H = 32.0                # conv1-activation fp8 scale
S_WP = 4096.0             # primary-caps weight fp8 scale
S_H2 = 2.0 ** 23          # h2 fp8 scale
S_W3 = 1024.0             # dec_w3 fp8 scale


def mkap(t, offset, dims):
    """Manual access pattern: dims = [[stride, count], ...] (partition dim first)."""
    return bass.AP(tensor=t.tensor if isinstance(t, bass.AP) else t, offset=offset, ap=dims)


def build_program():
    nc = bacc.Bacc(None, num_devices=NCORES)
    rg = [list(range(NCORES))]

    P = {}
    P["pat1h"] = nc.declare_dram_parameter("pat1h", [81, 4608], BF, isOutput=False)
    P["w1c"] = nc.declare_dram_parameter("w1c", [81, 256], BF, isOutput=False)
    P["b1s"] = nc.declare_dram_parameter("b1s", [256, 1], F32, isOutput=False)
    P["wp2q"] = nc.declare_dram_parameter("wp2q", [20736, 256], F8, isOutput=False)
    P["bp2"] = nc.declare_dram_parameter("bp2", [256, 1], F32, isOutput=False)
    P["wfull"] = nc.declare_dram_parameter("wfull", [128, 5120], BF, isOutput=False)
    P["ones8"] = nc.declare_dram_parameter("ones8", [128, 16], F32, isOutput=False)
    P["id128"] = nc.declare_dram_parameter("id128", [128, 128], BF, isOutput=False)
    P["w1t"] = nc.declare_dram_parameter("w1t", [160, 512], BF, isOutput=False)
    P["b1d"] = nc.declare_dram_parameter("b1d", [512, 1], F32, isOutput=False)
    P["w2t"] = nc.declare_dram_parameter("w2t", [512, 1024], BF, isOutput=False)
    P["b2s"] = nc.declare_dram_parameter("b2s", [1024, 1], F32, isOutput=False)
    P["w3q"] = nc.declare_dram_parameter("w3q", [128, 102400], F8, isOutput=False)
    P["b3q"] = nc.declare_dram_parameter("b3q", [1, RECL], BF, isOutput=False)
    P["onesrow"] = nc.declare_dram_parameter("onesrow", [1, 128], BF, isOutput=False)
    P["ones128"] = nc.declare_dram_parameter("ones128", [128, 1], F32, isOutput=False)
    out_ext = nc.declare_dram_parameter("out", [B, RECL], BF, isOutput=True)

    with tile.TileContext(nc) as tc:
        _body(nc, tc, P, out_ext, rg)
    nc.compile()
    return nc


def _body(nc, tc, P, out_ext, rg):
    es = tc.tile_pool(name="const", bufs=1)
    const = es.__enter__()
    dram_cm = tc.tile_pool(name="dram", bufs=1, space="DRAM")
    dram = dram_cm.__enter__()

    # ---------- DRAM scratch ----------
    warm_in = dram.tile([1, 16], F32, tag="warm_in", name="warm_in")
    warm_out = dram.tile([1, 16], F32, tag="warm_out", name="warm_out")
    xdram = dram.tile([2, 128, 512], BF, tag="xdram", name="xdram")
    warm2_in = dram.tile([16, 320], BF, tag="warm2_in", name="warm2_in")
    warm2_out = dram.tile([16, 320], BF, tag="warm2_out", name="warm2_out")
    ar_in = dram.tile([16, 320], BF, tag="ar_in", name="ar_in")
    ar_out = dram.tile([16, 320], BF, tag="ar_out", name="ar_out")
    c2d = dram.tile([10, 512], BF, tag="c2d", name="c2d")
    z_in = dram.tile([1, 16], F32, tag="z_in", name="z_in")
    z_out = dram.tile([1, 16], F32, tag="z_out", name="z_out")
    vin = dram.tile([BL, 160], F32, tag="vin", name="vin")
    vall = dram.tile([NCORES, BL, 160], F32, tag="vall", name="vall")

    # conv1 im2col patches: most critical load, goes first on the sync ring
    pat1 = const.tile([81, 4608], BF, tag="pat1", name="pat1")
    nc.sync.dma_start(pat1[:], P["pat1h"][:])

    # ---------- warmup collective (absorb first-collective setup cost) ----------
    zw = const.tile([1, 16], F32, tag="zw", name="zw")
    nc.vector.memset(zw[:], 0)
    nc.sync.dma_start(warm_in[:], zw[:])
    nc.gpsimd.collective_compute(
        "AllReduce", ALU.add, replica_groups=rg,
        ins=[warm_in[:].opt()], outs=[warm_out[:].opt()])
    zw2 = const.tile([16, 320], BF, tag="zw2", name="zw2")
    nc.vector.memset(zw2[:], 0)
    nc.sync.dma_start(warm2_in[:], zw2[:])
    nc.gpsimd.collective_compute(
        "AllReduce", ALU.add, replica_groups=rg,
        ins=[warm2_in[:].opt()], outs=[warm2_out[:].opt()])

    # ---------- constants to SBUF ----------
    w1c_sb = const.tile([81, 256], BF, tag="w1c", name="w1c")
    nc.sync.dma_start(w1c_sb[:], P["w1c"][:])
    b1s_sb = [const.tile([128, 1], F32, tag=f"b1s{h}", name=f"b1s{h}") for h in range(2)]
    bp2_sb = [const.tile([128, 1], F32, tag=f"bp2{h}", name=f"bp2{h}") for h in range(2)]
    for h in range(2):
        nc.sync.dma_start(b1s_sb[h][:], P["b1s"][h * 128:(h + 1) * 128, :])
        nc.sync.dma_start(bp2_sb[h][:], P["bp2"][h * 128:(h + 1) * 128, :])
    wfull_sb = const.tile([128, 5120], BF, tag="wfull", name="wfull")
    ones8_sb = const.tile([128, 16], F32, tag="ones8", name="ones8")
    nc.sync.dma_start(ones8_sb[:], P["ones8"][:])
    id128_sb = const.tile([128, 128], BF, tag="id128", name="id128")
    nc.sync.dma_start(id128_sb[:], P["id128"][:])
    w1ta_sb = const.tile([128, 512], BF, tag="w1ta", name="w1ta")
    nc.sync.dma_start(w1ta_sb[:], P["w1t"][0:128, :])
    w1tb_sb = const.tile([32, 512], BF, tag="w1tb", name="w1tb")
    nc.sync.dma_start(w1tb_sb[:], P["w1t"][128:160, :])
    b1d_sb = [const.tile([128, 1], F32, tag=f"b1d{i}", name=f"b1d{i}") for i in range(4)]
    for i in range(4):
        nc.sync.dma_start(b1d_sb[i][:], P["b1d"][i * 128:(i + 1) * 128, :])
    w2t_sb = [const.tile([128, 1024], BF, tag=f"w2t{i}", name=f"w2t{i}") for i in range(4)]
    b2s_sb = [const.tile([128, 1], F32, tag=f"b2s{i}", name=f"b2s{i}") for i in range(8)]
    for i in range(8):
        nc.sync.dma_start(b2s_sb[i][:], P["b2s"][i * 128:(i + 1) * 128, :])
    onesrow_sb = const.tile([1, 128], BF, tag="onesrow", name="onesrow")
    nc.sync.dma_start(onesrow_sb[:], P["onesrow"][:])
    ones128_sb = const.tile([128, 1], F32, tag="ones128", name="ones128")
    nc.sync.dma_start(ones128_sb[:], P["ones128"][:])
    b3q_sb = const.tile([1, RECL], BF, tag="b3q", name="b3q")
    w3q_sb = const.tile([128, 102400], F8, tag="w3q", name="w3q")

    # persistent mid-size tiles
    xT_sb = const.tile([128, 1024], BF, tag="xT", name="xT")      # [(cl,g,i), (chunk,b)]
    x2_sb = const.tile([BL, 4096], BF, tag="x2", name="x2")       # [b, (chunk,cl,g,i)]
    wc_sb = const.tile([128, 5120], BF, tag="wc", name="wc")      # c-weighted W / prod scratch
    h2q_sb = const.tile([128, 8, 256], F8, tag="h2q", name="h2q")  # [feat%128, kc, b]

    # =================== conv1 + primary caps (fp8 DoubleRow) ===================
    with tc.tile_pool(name="front", bufs=1) as front, \
         tc.tile_pool(name="wp2p", bufs=4) as wp2p, \
         tc.tile_pool(name="ps_f", bufs=2, space="PSUM") as ps_f:
        H = front.tile([128, 4608, 2], F8, tag="H", name="H")
        for h in range(2):
            for w in range(9):
                ps = ps_f.tile([128, 512], F32, tag="c1ps", name="c1ps")
                nc.tensor.matmul(ps[:], w1c_sb[:, h * 128:(h + 1) * 128],
                                 pat1[:, w * 512:(w + 1) * 512],
                                 start=True, stop=True)
                nc.scalar.activation(H[:, w * 512:(w + 1) * 512, h], ps[:],
                                     ACTF.Relu, bias=b1s_sb[h][:], scale=S_H)
        U = [front.tile([128, 512], F32, tag=f"U{h}", name=f"U{h}") for h in range(2)]
        psU = [ps_f.tile([128, 512], F32, tag=f"Ups{h}", name=f"Ups{h}", bufs=1) for h in range(2)]
        Hv = H[:].rearrange("p (y x b) c -> p c y x b", y=12, x=12)
        for u in range(81):
            dy, dx = divmod(u, 9)
            wt = wp2p.tile([128, 2, 256], F8, tag="wp2t", name="wp2t")
            nc.sync.dma_start(wt[:], mkap(P["wp2q"], u * 65536,
                                          [[256, 128], [32768, 2], [1, 256]]))
            rhs = Hv[:, :, dy:dy + 4, dx:dx + 4, :]
            for h in range(2):
                nc.tensor.matmul(psU[h][:], wt[:, :, h * 128:(h + 1) * 128], rhs,
                                 start=(u == 0), stop=(u == 80),
                                 perf_mode=PM.DoubleRow)
        for h in range(2):
            nc.scalar.activation(U[h][:], psU[h][:], ACTF.Identity,
                                 bias=bp2_sb[h][:], scale=1.0 / (S_H * S_WP))
        # big weight loads, delayed so they don't contend with the front stream
        nc.sync.dma_start(wfull_sb[:], P["wfull"][:])
        for i in range(4):
            nc.sync.dma_start(w2t_sb[i][:], P["w2t"][i * 128:(i + 1) * 128, :])
        nc.sync.dma_start(b3q_sb[:], P["b3q"][:])
        nc.scalar.dma_start(w3q_sb[:], P["w3q"][:])

        # ---- squash -> x (bf16), to DRAM, reload transposed ----
        usq = front.tile([128, 512], F32, tag="usq", name="usq")
        sn = front.tile([128, 64], F32, tag="sn", name="sn")
        g = front.tile([128, 64], F32, tag="g", name="g")
        gt = front.tile([128, 64], F32, tag="gt", name="gt")
        X = front.tile([128, 512], BF, tag="X", name="X")
        for h in range(2):
            nc.vector.tensor_tensor(usq[:], U[h][:], U[h][:], op=ALU.mult)
            uview = usq[:].rearrange("p (g i b) -> p g b i", g=2, i=8)
            nc.vector.tensor_reduce(sn[:].rearrange("p (g b) -> p g b", g=2),
                                    uview, axis=AX.X, op=ALU.add)
            nc.scalar.activation(gt[:], sn[:], ACTF.Sqrt)
            nc.vector.tensor_scalar_add(g[:], sn[:], 1.0)
            nc.vector.reciprocal(g[:], g[:])
            nc.vector.tensor_tensor(g[:], g[:], gt[:], op=ALU.mult)
            gb = g[:].rearrange("p (g b) -> p g b", g=2).unsqueeze(2).broadcast_to(
                [128, 2, 8, BL])
            nc.vector.tensor_tensor(X[:].rearrange("p (g i b) -> p g i b", g=2, i=8),
                                    U[h][:].rearrange("p (g i b) -> p g i b", g=2, i=8),
                                    gb, op=ALU.mult)
            nc.sync.dma_start(xdram[h], X[:])
        xsrc = mkap(xdram[:], 0, [[32, 128], [4096, 32], [1, 32]])
        nc.sync.dma_start(xT_sb[:], xsrc)

    nc.gpsimd.collective_compute(
        "AllReduce", ALU.add, replica_groups=rg,
        ins=[warm_in[:].opt()], outs=[warm_out[:].opt()])

    # =================== routing (3 iters, matmul-factored) ===================
    with tc.tile_pool(name="route", bufs=1) as rt, \
         tc.tile_pool(name="ps_r", bufs=1, space="PSUM") as ps_r:
        s_sb = rt.tile([BL, 160], F32, tag="s_sb", name="s_sb")
        sq = rt.tile([BL, 160], F32, tag="sq", name="sq")
        num = rt.tile([BL, 160], F32, tag="num", name="num")
        dn = rt.tile([BL, 160], F32, tag="dn", name="dn")
        v_sb = rt.tile([BL, 160], F32, tag="v_sb", name="v_sb")
        vq = rt.tile([BL, 160], BF, tag="vq", name="vq")
        t1 = rt.tile([128, 320], F32, tag="t1", name="t1")
        braw = rt.tile([16, 320], BF, tag="braw", name="braw")
        b_acc = rt.tile([16, 320], BF, tag="b_acc", name="b_acc")
        csf = rt.tile([10, 512], BF, tag="csf", name="csf")
        rmax = rt.tile([10, 1], F32, tag="rmax", name="rmax")
        nbias = rt.tile([10, 1], F32, tag="nbias", name="nbias")
        esum = rt.tile([10, 1], F32, tag="esum", name="esum")
        c_sb = rt.tile([10, 512], F32, tag="c_sb", name="c_sb")
        c_bf = rt.tile([10, 512], BF, tag="c_bf", name="c_bf")
        crep2 = rt.tile([128, 320], BF, tag="crep2", name="crep2")

        for it in range(3):
            # ---- s = sum_r c_r u_hat  via matmul over (r,i) ----
            psS = ps_r.tile([BL, 160], F32, tag="psS", name="psS")
            if it == 0:
                for j in range(32):
                    nc.tensor.matmul(psS[:], xT_sb[:, j * 32:(j + 1) * 32],
                                     wfull_sb[:, j * 160:(j + 1) * 160],
                                     start=(j == 0), stop=(j == 31))
                nc.vector.tensor_scalar(s_sb[:], psS[:], 1.0 / 512.0, None, op0=ALU.mult)
            else:
                cvv = crep2[:].rearrange("p (c m) -> p m c", c=10).unsqueeze(3)
                wfv = wfull_sb[:].rearrange("p (m c o) -> p m c o", m=32, c=10)
                wcv = wc_sb[:].rearrange("p (m c o) -> p m c o", m=32, c=10)
                for seg in range(4):
                    sl = slice(8 * seg, 8 * (seg + 1))
                    nc.vector.tensor_tensor(
                        wcv[:, sl], wfv[:, sl],
                        cvv[:, sl].broadcast_to([128, 8, 10, 16]), op=ALU.mult)
                    for j in range(8 * seg, 8 * (seg + 1)):
                        nc.tensor.matmul(psS[:], xT_sb[:, j * 32:(j + 1) * 32],
                                         wc_sb[:, j * 160:(j + 1) * 160],
                                         start=(j == 0), stop=(j == 31))
                nc.vector.tensor_copy(s_sb[:], psS[:])
            # ---- elementwise squash: v = sq*s/((1+sq)*sqrt(sq)) ----
            nc.vector.tensor_tensor(sq[:], s_sb[:], s_sb[:], op=ALU.mult)
            nc.vector.tensor_tensor(num[:], sq[:], s_sb[:], op=ALU.mult)
            nc.vector.tensor_scalar_add(dn[:], sq[:], 1.0)
            nc.scalar.activation(sq[:], sq[:], ACTF.Sqrt)
            nc.vector.tensor_tensor(dn[:], dn[:], sq[:], op=ALU.mult)
            nc.vector.reciprocal(dn[:], dn[:])
            nc.vector.tensor_tensor(v_sb[:], num[:], dn[:], op=ALU.mult)

            if it == 2:
                break
            if it == 0:
                # x2 via 32 tensor transposes of xT chunks (overlaps squash)
                for j in range(32):
                    psT = ps_r.tile([32, 128], BF, tag="psT", name="psT", bufs=2)
                    nc.tensor.transpose(psT[:], xT_sb[:, j * 32:(j + 1) * 32], id128_sb[:])
                    nc.vector.tensor_copy(x2_sb[:, j * 128:(j + 1) * 128], psT[:])
            # ---- G[(r,i),(cd,o)] = sum_b x v  (32 matmuls) ----
            nc.vector.tensor_copy(vq[:], v_sb[:])
            for j in range(32):
                psG = ps_r.tile([128, 160], F32, tag="psG", name="psG", bufs=2)
                nc.tensor.matmul(psG[:], x2_sb[:, j * 128:(j + 1) * 128], vq[:],
                                 start=True, stop=True)
                nc.vector.tensor_tensor(wc_sb[:, j * 160:(j + 1) * 160], psG[:],
                                        wfull_sb[:, j * 160:(j + 1) * 160], op=ALU.mult)
            # ---- a_mean = sum_{o,i} W . G ----
            nc.vector.tensor_reduce(
                t1[:].rearrange("p (m c) -> p m c", m=32),
                wc_sb[:].rearrange("p (m c o) -> p m c o", m=32, c=10),
                axis=AX.X, op=ALU.add)
            psA = ps_r.tile([16, 320], F32, tag="psA", name="psA")
            nc.tensor.matmul(psA[:], ones8_sb[:],
                             t1[:].rearrange("p (m c) -> p c m", m=32),
                             start=True, stop=True)
            # fold previous b-state into the reduce: AR output IS the new b
            if it == 0:
                nc.vector.tensor_copy(braw[:], psA[:])
            else:
                nc.vector.scalar_tensor_tensor(braw[:], b_acc[:], 0.125, psA[:],
                                               op0=ALU.mult, op1=ALU.add)
            nc.sync.dma_start(ar_in[:], braw[:])
            nc.gpsimd.collective_compute(
                "AllReduce", ALU.add, replica_groups=rg,
                ins=[ar_in[:].opt()], outs=[ar_out[:].opt()])
            # ---- softmax over routes: b [16,(cd,m)] -> csf [10,(q,m)] ----
            nc.sync.dma_start(csf[:], mkap(ar_out[:], 0, [[32, 10], [320, 16], [1, 32]]))
            nc.gpsimd.dma_start(b_acc[:], ar_out[:])
            nc.vector.tensor_reduce(rmax[:], csf[:], axis=AX.X, op=ALU.max)
            nc.scalar.mul(nbias[:], rmax[:], -1.0)
            nc.scalar.activation(c_sb[:], csf[:], ACTF.Exp, bias=nbias[:], scale=1.0)
            nc.vector.tensor_reduce(esum[:], c_sb[:], axis=AX.X, op=ALU.add)
            nc.vector.reciprocal(esum[:], esum[:])
            nc.vector.tensor_scalar_mul(c_sb[:], c_sb[:], esum[:])
            nc.vector.tensor_copy(c_bf[:], c_sb[:])
            nc.sync.dma_start(c2d[:], c_bf[:])
            for q in range(16):
                src = mkap(c2d[:], q * 32, [[0, 8], [512, 10], [1, 32]])
                eng = nc.sync if q % 2 == 0 else nc.gpsimd
                eng.dma_start(crep2[8 * q:8 * (q + 1), :], src)

        # =================== tail: AllGather v, full-batch decoder ===========
        nc.sync.dma_start(vin[:], v_sb[:])
        nc.gpsimd.collective_compute(
            "AllGather", ALU.bypass, replica_groups=rg,
            ins=[vin[:].opt()], outs=[vall[:].opt()])

        vfull = [rt.tile([128, 160], F32, tag=f"vf{bh}", name=f"vf{bh}") for bh in range(2)]
        ecl = [rt.tile([128, 10], F32, tag=f"ecl{bh}", name=f"ecl{bh}") for bh in range(2)]
        sqf = rt.tile([128, 160], F32, tag="sqf", name="sqf")
        cl = rt.tile([128, 10], F32, tag="cl", name="cl")
        psZ = ps_r.tile([1, 16], F32, tag="psA", name="psZ", bufs=1)
        for bh in range(2):
            nc.sync.dma_start(vfull[bh][:],
                              mkap(vall[:], bh * 128 * 160, [[160, 128], [1, 160]]))
            nc.vector.tensor_tensor(sqf[:], vfull[bh][:], vfull[bh][:], op=ALU.mult)
            nc.vector.tensor_reduce(cl[:], sqf[:].rearrange("p (c o) -> p c o", c=10),
                                    axis=AX.X, op=ALU.add)
            nc.scalar.activation(cl[:], cl[:], ACTF.Sqrt)
            nc.scalar.activation(ecl[bh][:], cl[:], ACTF.Exp)
            nc.tensor.matmul(psZ[:, :10], ones128_sb[:], ecl[bh][:],
                             start=(bh == 0), stop=(bh == 1))
        zrow = rt.tile([1, 16], F32, tag="zrow", name="zrow")
        nc.vector.memset(zrow[:], 0)
        nc.vector.tensor_copy(zrow[:, :10], psZ[:, :10])
        nc.vector.reciprocal(zrow[:, :10], zrow[:, :10])
        nc.sync.dma_start(z_in[:], zrow[:])
        zfull = rt.tile([128, 10], F32, tag="zfull", name="zfull")
        nc.gpsimd.dma_start(zfull[:], mkap(z_in[:], 0, [[0, 128], [1, 10]]))

        tpr = rt.tile([128, 10], F32, tag="tpr", name="tpr")
        tmax = rt.tile([128, 1], F32, tag="tmax", name="tmax")
        mask = rt.tile([128, 10], F32, tag="mask", name="mask")
        flat = rt.tile([128, 160], BF, tag="flat", name="flat")
        flatTa = rt.tile([128, 256], BF, tag="flatTa", name="flatTa")
        flatTb = rt.tile([32, 256], BF, tag="flatTb", name="flatTb")
        h1q = [rt.tile([128, 256], BF, tag=f"h1q{i}", name=f"h1q{i}") for i in range(4)]
        for bh in range(2):
            nc.vector.tensor_tensor(tpr[:], ecl[bh][:], zfull[:], op=ALU.mult)
            nc.vector.tensor_reduce(tmax[:], tpr[:], axis=AX.X, op=ALU.max)
            nc.vector.tensor_scalar(mask[:], tpr[:], tmax[:], None, op0=ALU.is_equal)
            mb = mask[:].unsqueeze(2).broadcast_to([128, 10, 16])
            nc.vector.tensor_tensor(flat[:].rearrange("p (c o) -> p c o", c=10),
                                    vfull[bh][:].rearrange("p (c o) -> p c o", c=10),
                                    mb, op=ALU.mult)
            psT1 = ps_r.tile([128, 128], BF, tag="psT", name="psT1", bufs=2)
            nc.tensor.transpose(psT1[:], flat[:, 0:128], id128_sb[:])
            nc.vector.tensor_copy(flatTa[:, bh * 128:(bh + 1) * 128], psT1[:])
            psT2 = ps_r.tile([32, 128], BF, tag="psT", name="psT2", bufs=2)
            nc.tensor.transpose(psT2[:], flat[:, 128:160], id128_sb[:])
            nc.vector.tensor_copy(flatTb[:, bh * 128:(bh + 1) * 128], psT2[:])
        # fc1: h1 = relu(w1 @ flat + b1)   [512, 256]
        for fc in range(4):
            ps1 = ps_r.tile([128, 256], F32, tag="psD", name="ps1", bufs=2)
            nc.tensor.matmul(ps1[:], w1ta_sb[:, fc * 128:(fc + 1) * 128], flatTa[:],
                             start=True, stop=False)
            nc.tensor.matmul(ps1[:], w1tb_sb[:, fc * 128:(fc + 1) * 128], flatTb[:],
                             start=False, stop=True)
            nc.scalar.activation(h1q[fc][:], ps1[:], ACTF.Relu, bias=b1d_sb[fc][:],
                                 scale=1.0)
        # fc2: h2 = relu(w2 @ h1 + b2), quantized to fp8 * S_H2
        for gc in range(8):
            ps2 = ps_r.tile([128, 256], F32, tag="psD", name="ps2", bufs=2)
            for kc in range(4):
                nc.tensor.matmul(ps2[:], w2t_sb[kc][:, gc * 128:(gc + 1) * 128],
                                 h1q[kc][:], start=(kc == 0), stop=(kc == 3))
            nc.scalar.activation(h2q_sb[:, gc, :], ps2[:], ACTF.Relu,
                                 bias=b2s_sb[gc][:], scale=S_H2)

    # =================== final big layer (fp8 DoubleRow, weights resident) ====
    with tc.tile_pool(name="ps_o", bufs=4, space="PSUM") as ps_o, \
         tc.tile_pool(name="osb", bufs=4) as osbp:
        w3v = w3q_sb[:].rearrange("p (w r n j) -> p w r j n", w=NW, r=4, j=2)
        for w in range(NW):
            for bh in range(2):
                pso = ps_o.tile([128, 512], F32, tag="pso", name="pso")
                for pr in range(4):
                    nc.tensor.matmul(pso[:],
                                     h2q_sb[:, 2 * pr:2 * pr + 2, bh * 128:(bh + 1) * 128],
                                     w3v[:, w, pr], start=(pr == 0), stop=False,
                                     perf_mode=PM.DoubleRow)
                nc.tensor.matmul(pso[:], onesrow_sb[:],
                                 b3q_sb[:, w * 512:(w + 1) * 512],
                                 start=False, stop=True)
                ot = osbp.tile([128, 512], BF, tag="ot", name="ot")
                nc.scalar.activation(ot[:], pso[:], ACTF.Sigmoid, scale=1.0 / (S_H2 * S_W3))
                nc.sync.dma_start(out_ext[bh * 128:(bh + 1) * 128,
                                          w * 512:(w + 1) * 512], ot[:])


_NC_CACHE = {}


def _host_prep(inputs):
    data = np.asarray(inputs["data"], np.float32)
    conv1_w = np.asarray(inputs["conv1_w"], np.float32)
    conv1_b = np.asarray(inputs["conv1_b"], np.float32)
    prim_w = np.asarray(inputs["prim_w"], np.float32)
    prim_b = np.asarray(inputs["prim_b"], np.float32)
    W_digit = np.asarray(inputs["W_digit"], np.float32)
    dec_w1 = np.asarray(inputs["dec_w1"], np.float32)
    dec_b1 = np.asarray(inputs["dec_b1"], np.float32)
    dec_w2 = np.asarray(inputs["dec_w2"], np.float32)
    dec_b2 = np.asarray(inputs["dec_b2"], np.float32)
    dec_w3 = np.asarray(inputs["dec_w3"], np.float32)
    dec_b3 = np.asarray(inputs["dec_b3"], np.float32)

    w1c = np.ascontiguousarray(conv1_w[:, 0].transpose(1, 2, 0).reshape(81, 256)).astype(BF16)
    wp2q = np.ascontiguousarray(
        prim_w.transpose(2, 3, 1, 0).reshape(20736, 256) * S_WP).astype(FP8)
    # Wfull [p=(cl,g,i), (chunk(h,cc), cd, o)]; route r = 256h + 16cc + 2cl + g
    Wv = W_digit.reshape(2, 16, 8, 2, 10, 16, 8)  # [h, cc, cl, g, cd, o, i]
    wfull = np.ascontiguousarray(Wv.transpose(2, 3, 6, 0, 1, 4, 5)).reshape(128, 5120).astype(BF16)
    ones8 = np.zeros((128, 16), np.float32)
    ones8[np.arange(128), np.arange(128) // 8] = 1.0 / 256.0
    w1t = np.ascontiguousarray(dec_w1.T).astype(BF16)
    w2t = np.ascontiguousarray(dec_w2.T).astype(BF16)
    w3t = np.ascontiguousarray(dec_w3.T)  # [1024, 102400]

    common = dict(
        w1c=w1c, b1s=(conv1_b * S_H).reshape(256, 1),
        bp2=prim_b.reshape(256, 1), wp2q=wp2q, wfull=wfull,
        ones8=ones8, id128=np.eye(128, dtype=np.float32).astype(BF16),
        w1t=w1t, b1d=dec_b1.reshape(512, 1),
        w2t=w2t, b2s=(dec_b2 * S_H2).reshape(1024, 1),
        onesrow=np.ones((1, 128), np.float32).astype(BF16),
        ones128=np.ones((128, 1), np.float32),
    )
    in_maps = []
    for c in range(NCORES):
        m = dict(common)
        sw = np.lib.stride_tricks.sliding_window_view(
            data[c * BL:(c + 1) * BL, 0], (9, 9), axis=(1, 2))
        m["pat1h"] = np.ascontiguousarray(
            sw.transpose(3, 4, 1, 2, 0).reshape(81, 4608)).astype(BF16)
        w3c = w3t[:, c * RECL:(c + 1) * RECL] * S_W3   # [1024, 12800]
        m["w3q"] = np.ascontiguousarray(
            w3c.reshape(4, 2, 128, NW, 512).transpose(2, 3, 0, 4, 1).reshape(128, 102400)
        ).astype(FP8)
        m["b3q"] = (dec_b3[c * RECL:(c + 1) * RECL] * (S_H2 * S_W3)).reshape(1, RECL).astype(BF16)
        in_maps.append(m)
    return in_maps


def kernel(**inputs):
    if "nc" not in _NC_CACHE:
        _NC_CACHE["nc"] = build_program()
    nc = _NC_CACHE["nc"]
    in_maps = _host_prep(inputs)
    res = run_bass_kernel_spmd(nc, in_maps, list(range(NCORES)))
    outs = [np.asarray(res.results[c]["out"]).astype(np.float32) for c in range(NCORES)]
    rec = np.concatenate(outs, axis=1)
    return rec.reshape(B, 256, 20, 20)


# revision 20
# speedup vs baseline: 1.2858x; 1.2858x over previous
import sys

sys.path.insert(0, "/opt/trn_rl_repo")

import numpy as np
import ml_dtypes

import concourse.bass as bass
import concourse.mybir as mybir
import concourse.tile as tile
from concourse import bacc
from concourse.bass_utils import run_bass_kernel_spmd

BF16 = ml_dtypes.bfloat16
FP8 = ml_dtypes.float8_e4m3
F32 = mybir.dt.float32
BF = mybir.dt.bfloat16
F8 = mybir.dt.float8e4
ALU = mybir.AluOpType
ACTF = mybir.ActivationFunctionType
AX = mybir.AxisListType
PM = mybir.MatmulPerfMode

NCORES = 8
B = 256
BL = B // NCORES          # 32 local batch
REC = 102400
RECL = REC // NCORES      # 12800 local output cols
NW = RECL // 512          # 25 output windows

S_H = 32.0                # conv1-activation fp8 scale
S_WP = 4096.0             # primary-caps weight fp8 scale
S_H2 = 2.0 ** 23          # h2 fp8 scale
S_W3 = 1024.0             # dec_w3 fp8 scale


def mkap(t, offset, dims):
    """Manual access pattern: dims = [[stride, count], ...] (partition dim first)."""
    return bass.AP(tensor=t.tensor if isinstance(t, bass.AP) else t, offset=offset, ap=dims)


def build_program():
    nc = bacc.Bacc(None, num_devices=NCORES)
    rg = [list(range(NCORES))]

    P = {}
    out_ext = nc.declare_dram_parameter("out", [B, RECL], BF, isOutput=True)
    P["w3q"] = nc.declare_dram_parameter("w3q", [128, 102400], F8, isOutput=False)
    P["w2t"] = nc.declare_dram_parameter("w2t", [512, 1024], BF, isOutput=False)
    P["wfull"] = nc.declare_dram_parameter("wfull", [128, 5120], BF, isOutput=False)
    P["b3q"] = nc.declare_dram_parameter("b3q", [1, RECL], BF, isOutput=False)
    P["w1t"] = nc.declare_dram_parameter("w1t", [160, 512], BF, isOutput=False)
    P["b1d"] = nc.declare_dram_parameter("b1d", [512, 1], F32, isOutput=False)
    P["b2s"] = nc.declare_dram_parameter("b2s", [1024, 1], F32, isOutput=False)
    P["id128"] = nc.declare_dram_parameter("id128", [128, 128], BF, isOutput=False)
    P["id128f"] = nc.declare_dram_parameter("id128f", [128, 128], F32, isOutput=False)
    P["onesrow"] = nc.declare_dram_parameter("onesrow", [1, 128], BF, isOutput=False)
    P["ones128"] = nc.declare_dram_parameter("ones128", [128, 1], F32, isOutput=False)
    P["wp2q"] = nc.declare_dram_parameter("wp2q", [20736, 256], F8, isOutput=False)
    P["bp2"] = nc.declare_dram_parameter("bp2", [256, 1], F32, isOutput=False)
    P["b1s"] = nc.declare_dram_parameter("b1s", [256, 1], F32, isOutput=False)
    P["w1c"] = nc.declare_dram_parameter("w1c", [81, 256], BF, isOutput=False)
    P["pat1h"] = nc.declare_dram_parameter("pat1h", [81, 4608], BF, isOutput=False)

    with tile.TileContext(nc) as tc:
        _body(nc, tc, P, out_ext, rg)
    nc.compile()
    return nc


def _body(nc, tc, P, out_ext, rg):
    es = tc.tile_pool(name="const", bufs=1)
    const = es.__enter__()
    dram_cm = tc.tile_pool(name="dram", bufs=1, space="DRAM")
    dram = dram_cm.__enter__()

    # ---------- DRAM scratch ----------
    warm_in = dram.tile([1, 16], F32, tag="warm_in", name="warm_in")
    warm_out = dram.tile([1, 16], F32, tag="warm_out", name="warm_out")
    xdram = dram.tile([2, 128, 512], BF, tag="xdram", name="xdram")
    warm2_in = dram.tile([40, 128], F32, tag="warm2_in", name="warm2_in")
    warm2_out = dram.tile([40, 128], F32, tag="warm2_out", name="warm2_out")
    ar_in = dram.tile([40, 128], F32, tag="ar_in", name="ar_in")
    ar_out = dram.tile([40, 128], F32, tag="ar_out", name="ar_out")
    c2d = dram.tile([10, 512], BF, tag="c2d", name="c2d")
    z_in = dram.tile([1, 16], F32, tag="z_in", name="z_in")
    z_out = dram.tile([1, 16], F32, tag="z_out", name="z_out")
    vin = dram.tile([BL, 160], F32, tag="vin", name="vin")
    vall = dram.tile([NCORES, BL, 160], F32, tag="vall", name="vall")

    # conv1 im2col patches: most critical load, on the gpsimd ring (less
    # contended by the input-upload traffic at kernel start)
    pat1 = const.tile([81, 4608], BF, tag="pat1", name="pat1")
    nc.gpsimd.dma_start(pat1[:], P["pat1h"][:])

    # ---------- constants to SBUF ----------
    w1c_sb = const.tile([81, 256], BF, tag="w1c", name="w1c")
    nc.gpsimd.dma_start(w1c_sb[:], P["w1c"][:])
    # warmup collectives (absorb first-collective setup cost); queued on
    # gpsimd after the two critical front loads
    zw = const.tile([1, 16], F32, tag="zw", name="zw")
    nc.vector.memset(zw[:], 0)
    nc.sync.dma_start(warm_in[:], zw[:])
    nc.gpsimd.collective_compute(
        "AllReduce", ALU.add, replica_groups=rg,
        ins=[warm_in[:].opt()], outs=[warm_out[:].opt()])
    zw2 = const.tile([40, 128], F32, tag="zw2", name="zw2")
    nc.vector.memset(zw2[:], 0)
    nc.sync.dma_start(warm2_in[:], zw2[:])
    nc.gpsimd.collective_compute(
        "AllReduce", ALU.add, replica_groups=rg,
        ins=[warm2_in[:].opt()], outs=[warm2_out[:].opt()])
    b1s_sb = [const.tile([128, 1], F32, tag=f"b1s{h}", name=f"b1s{h}") for h in range(2)]
    bp2_sb = [const.tile([128, 1], F32, tag=f"bp2{h}", name=f"bp2{h}") for h in range(2)]
    for h in range(2):
        nc.sync.dma_start(b1s_sb[h][:], P["b1s"][h * 128:(h + 1) * 128, :])
        nc.sync.dma_start(bp2_sb[h][:], P["bp2"][h * 128:(h + 1) * 128, :])
    wfull_sb = const.tile([128, 5120], BF, tag="wfull", name="wfull")
    id128_sb = const.tile([128, 128], BF, tag="id128", name="id128")
    nc.sync.dma_start(id128_sb[:], P["id128"][:])
    id128f_sb = const.tile([128, 128], F32, tag="id128f", name="id128f")
    nc.sync.dma_start(id128f_sb[:], P["id128f"][:])
    w1ta_sb = const.tile([128, 512], BF, tag="w1ta", name="w1ta")
    nc.sync.dma_start(w1ta_sb[:], P["w1t"][0:128, :])
    w1tb_sb = const.tile([32, 512], BF, tag="w1tb", name="w1tb")
    nc.sync.dma_start(w1tb_sb[:], P["w1t"][128:160, :])
    b1d_sb = [const.tile([128, 1], F32, tag=f"b1d{i}", name=f"b1d{i}") for i in range(4)]
    for i in range(4):
        nc.sync.dma_start(b1d_sb[i][:], P["b1d"][i * 128:(i + 1) * 128, :])
    w2t_sb = [const.tile([128, 1024], BF, tag=f"w2t{i}", name=f"w2t{i}") for i in range(4)]
    b2s_sb = [const.tile([128, 1], F32, tag=f"b2s{i}", name=f"b2s{i}") for i in range(8)]
    for i in range(8):
        nc.sync.dma_start(b2s_sb[i][:], P["b2s"][i * 128:(i + 1) * 128, :])
    onesrow_sb = const.tile([1, 128], BF, tag="onesrow", name="onesrow")
    nc.sync.dma_start(onesrow_sb[:], P["onesrow"][:])
    ones128_sb = const.tile([128, 1], F32, tag="ones128", name="ones128")
    nc.sync.dma_start(ones128_sb[:], P["ones128"][:])
    b3q_sb = const.tile([1, RECL], BF, tag="b3q", name="b3q")

    # persistent mid-size tiles
    X = [const.tile([128, 512], BF, tag=f"X{h}", name=f"X{h}") for h in range(2)]
    x2_sb = const.tile([BL, 4096], BF, tag="x2", name="x2")       # [b, (j,co)]
    wc_sb = const.tile([128, 5120], BF, tag="wc", name="wc")      # c-weighted W / prod scratch
    h2q_sb = const.tile([128, 8, 256], F8, tag="h2q", name="h2q")  # [feat%128, kc, b]

    # =================== conv1 + primary caps (fp8 DoubleRow) ===================
    with tc.tile_pool(name="front", bufs=1) as front, \
         tc.tile_pool(name="ps_f", bufs=2, space="PSUM") as ps_f:
        H = front.tile([128, 4608, 2], F8, tag="H", name="H")
        # full primary-caps weight, preloaded (k-pairs in dim1)
        wp2q_sb = front.tile([128, 162, 256], F8, tag="wp2q", name="wp2q")
        nc.sync.dma_start(wp2q_sb[:], mkap(P["wp2q"], 0, [[256, 128], [32768, 162], [1, 256]]))
        for h in range(2):
            for w in range(9):
                ps = ps_f.tile([128, 512], F32, tag="c1ps", name="c1ps")
                nc.tensor.matmul(ps[:], w1c_sb[:, h * 128:(h + 1) * 128],
                                 pat1[:, w * 512:(w + 1) * 512],
                                 start=True, stop=True)
                nc.scalar.activation(H[:, w * 512:(w + 1) * 512, h], ps[:],
                                     ACTF.Relu, bias=b1s_sb[h][:], scale=S_H)
        U = [front.tile([128, 512], F32, tag=f"U{h}", name=f"U{h}") for h in range(2)]
        psU = [ps_f.tile([128, 512], F32, tag=f"Ups{h}", name=f"Ups{h}", bufs=1) for h in range(2)]
        Hv = H[:].rearrange("p (y x b) c -> p c y x b", y=12, x=12)
        for h in range(2):
            for u in range(81):
                dy, dx = divmod(u, 9)
                rhs = Hv[:, :, dy:dy + 4, dx:dx + 4, :]
                nc.tensor.matmul(psU[h][:],
                                 wp2q_sb[:, 2 * u:2 * u + 2, h * 128:(h + 1) * 128],
                                 rhs, start=(u == 0), stop=(u == 80),
                                 perf_mode=PM.DoubleRow)
        for h in range(2):
            nc.scalar.activation(U[h][:], psU[h][:], ACTF.Identity,
                                 bias=bp2_sb[h][:], scale=1.0 / (S_H * S_WP))
        # big weight loads, delayed so they don't contend with the front stream
        nc.sync.dma_start(wfull_sb[:], P["wfull"][:])
        for i in range(4):
            nc.sync.dma_start(w2t_sb[i][:], P["w2t"][i * 128:(i + 1) * 128, :])
        nc.sync.dma_start(b3q_sb[:], P["b3q"][:])

        # ---- squash -> x (bf16), to DRAM, reload transposed ----
        usq = front.tile([128, 512], F32, tag="usq", name="usq")
        sn = front.tile([128, 64], F32, tag="sn", name="sn")
        g = front.tile([128, 64], F32, tag="g", name="g")
        gt = front.tile([128, 64], F32, tag="gt", name="gt")
        for h in range(2):
            nc.vector.tensor_tensor(usq[:], U[h][:], U[h][:], op=ALU.mult)
            uview = usq[:].rearrange("p (g i b) -> p g b i", g=2, i=8)
            nc.vector.tensor_reduce(sn[:].rearrange("p (g b) -> p g b", g=2),
                                    uview, axis=AX.X, op=ALU.add)
            nc.scalar.activation(gt[:], sn[:], ACTF.Sqrt)
            nc.vector.tensor_scalar_add(g[:], sn[:], 1.0)
            nc.vector.reciprocal(g[:], g[:])
            nc.vector.tensor_tensor(g[:], g[:], gt[:], op=ALU.mult)
            gb = g[:].rearrange("p (g b) -> p g b", g=2).unsqueeze(2).broadcast_to(
                [128, 2, 8, BL])
            nc.vector.tensor_tensor(X[h][:].rearrange("p (g i b) -> p g i b", g=2, i=8),
                                    U[h][:].rearrange("p (g i b) -> p g i b", g=2, i=8),
                                    gb, op=ALU.mult)

    nc.gpsimd.collective_compute(
        "AllReduce", ALU.add, replica_groups=rg,
        ins=[warm_in[:].opt()], outs=[warm_out[:].opt()])

    w3cm = tc.tile_pool(name="w3pool", bufs=1)
    w3pool = w3cm.__enter__()
    w3q_sb = w3pool.tile([128, 102400], F8, tag="w3q", name="w3q")
    nc.scalar.dma_start(w3q_sb[:], P["w3q"][:])

    # =================== routing (3 iters, matmul-factored) ===================
    with tc.tile_pool(name="route", bufs=1) as rt, \
         tc.tile_pool(name="ps_r", bufs=1, space="PSUM") as ps_r:
        s_sb = rt.tile([BL, 160], F32, tag="s_sb", name="s_sb")
        sq = rt.tile([BL, 160], F32, tag="sq", name="sq")
        num = rt.tile([BL, 160], F32, tag="num", name="num")
        dn = rt.tile([BL, 160], F32, tag="dn", name="dn")
        v_sb = rt.tile([BL, 160], F32, tag="v_sb", name="v_sb")
        vq = rt.tile([BL, 160], BF, tag="vq", name="vq")
        arin_sb = rt.tile([40, 128], F32, tag="arin_sb", name="arin_sb")
        b_accT = rt.tile([40, 128], F32, tag="b_accT", name="b_accT")
        braw = rt.tile([128, 40], F32, tag="braw", name="braw")
        csf = rt.tile([10, 512], F32, tag="csf", name="csf")
        rmax = rt.tile([10, 1], F32, tag="rmax", name="rmax")
        nbias = rt.tile([10, 1], F32, tag="nbias", name="nbias")
        esum = rt.tile([10, 1], F32, tag="esum", name="esum")
        c_sb = rt.tile([10, 512], F32, tag="c_sb", name="c_sb")
        cT = rt.tile([128, 40], F32, tag="cT", name="cT")

        def xsl(j):
            h, gg, ii = j >> 4, (j >> 3) & 1, j & 7
            return X[h][:].rearrange("p (g i b) -> p g i b", g=2, i=8)[:, gg, ii, :]

        for it in range(3):
            # ---- s[b,cd,o] = sum_{r,i} c.W.x  via 32 accumulated matmuls ----
            psS = ps_r.tile([BL, 160], F32, tag="psS", name="psS")
            if it == 0:
                for j in range(32):
                    nc.tensor.matmul(psS[:], xsl(j),
                                     wfull_sb[:, j * 160:(j + 1) * 160],
                                     start=(j == 0), stop=(j == 31))
                nc.vector.tensor_scalar(s_sb[:], psS[:], 1.0 / 512.0, None, op0=ALU.mult)
            else:
                cvv = cT[:].rearrange("p (hg c) -> p hg c", hg=4).unsqueeze(2)\
                    .unsqueeze(4)
                wfv = wfull_sb[:].rearrange("p (hg i c o) -> p hg i c o", hg=4, i=8, c=10)
                wcv = wc_sb[:].rearrange("p (hg i c o) -> p hg i c o", hg=4, i=8, c=10)
                for seg in range(4):
                    nc.vector.tensor_tensor(
                        wcv[:, seg], wfv[:, seg],
                        cvv[:, seg].broadcast_to([128, 8, 10, 16]), op=ALU.mult)
                    for j in range(8 * seg, 8 * (seg + 1)):
                        nc.tensor.matmul(psS[:], xsl(j),
                                         wc_sb[:, j * 160:(j + 1) * 160],
                                         start=(j == 0), stop=(j == 31))
                nc.vector.tensor_copy(s_sb[:], psS[:])
            # ---- elementwise squash: v = sq*s/((1+sq)*sqrt(sq)) ----
            nc.vector.tensor_tensor(sq[:], s_sb[:], s_sb[:], op=ALU.mult)
            nc.vector.tensor_tensor(num[:], sq[:], s_sb[:], op=ALU.mult)
            nc.vector.tensor_scalar_add(dn[:], sq[:], 1.0)
            nc.scalar.activation(sq[:], sq[:], ACTF.Sqrt)
            nc.vector.tensor_tensor(dn[:], dn[:], sq[:], op=ALU.mult)
            nc.vector.reciprocal(dn[:], dn[:])
            nc.vector.tensor_tensor(v_sb[:], num[:], dn[:], op=ALU.mult)

            if it == 2:
                break
            if it == 0:
                # x2[b, (j,co)] via 32 tensor transposes of X slices
                for j in range(32):
                    psT = ps_r.tile([32, 128], BF, tag="psT", name="psT", bufs=2)
                    nc.tensor.transpose(psT[:], xsl(j), id128_sb[:])
                    nc.vector.tensor_copy(x2_sb[:, j * 128:(j + 1) * 128], psT[:])
            # ---- G[(r,i),(cd,o)] = sum_b x v; prod = G.W fused from psum ----
            nc.vector.tensor_copy(vq[:], v_sb[:])
            for j in range(32):
                psG = ps_r.tile([128, 160], F32, tag="psG", name="psG", bufs=2)
                nc.tensor.matmul(psG[:], x2_sb[:, j * 128:(j + 1) * 128], vq[:],
                                 start=True, stop=True)
                nc.vector.tensor_tensor(wc_sb[:, j * 160:(j + 1) * 160], psG[:],
                                        wfull_sb[:, j * 160:(j + 1) * 160], op=ALU.mult)
            # ---- a_mean[r,cd] = sum_{i,o} prod : one strided XY reduce ----
            prv = wc_sb[:].rearrange("p (h g i c o) -> p h g c i o", h=2, g=2, i=8, c=10, o=16)
            nc.vector.tensor_reduce(
                braw[:].rearrange("p (c h g) -> p h g c", c=10, h=2),
                prv, axis=AX.XY, op=ALU.add)
            # transpose to [(cd,h,g), co] and fold previous b-state (/256 batch,
            # /8 cores so the AllReduce sum is the new b directly)
            psB = ps_r.tile([40, 128], F32, tag="psA", name="psB")
            nc.tensor.transpose(psB[:], braw[:], id128f_sb[:])
            if it == 0:
                nc.vector.tensor_scalar(arin_sb[:], psB[:], 1.0 / 256.0, None, op0=ALU.mult)
            else:
                nc.vector.scalar_tensor_tensor(arin_sb[:], psB[:], 1.0 / 256.0,
                                               b_accT[:], op0=ALU.mult, op1=ALU.add)
            nc.sync.dma_start(ar_in[:], arin_sb[:])
            nc.gpsimd.collective_compute(
                "AllReduce", ALU.add, replica_groups=rg,
                ins=[ar_in[:].opt()], outs=[ar_out[:].opt()])
            # b_accT := b_new/8  (next round each core contributes b/8 so the
            # 8-way sum reconstructs b)
            nc.sync.dma_start(csf[:], mkap(ar_out[:], 0,
                                           [[512, 10], [256, 2], [128, 2], [1, 128]]))
            nc.gpsimd.dma_start(b_accT[:], ar_out[:])
            nc.vector.tensor_scalar(b_accT[:], b_accT[:], 0.125, None, op0=ALU.mult)
            # ---- softmax over routes (free dim) ----
            nc.vector.tensor_reduce(rmax[:], csf[:], axis=AX.X, op=ALU.max)
            nc.scalar.mul(nbias[:], rmax[:], -1.0)
            nc.scalar.activation(c_sb[:], csf[:], ACTF.Exp, bias=nbias[:], scale=1.0)
            nc.vector.tensor_reduce(esum[:], c_sb[:], axis=AX.X, op=ALU.add)
            nc.vector.reciprocal(esum[:], esum[:])
            nc.vector.tensor_scalar_mul(c_sb[:], c_sb[:], esum[:])
            # ---- cT[co, (h,g,cd)] via 4 on-chip transposes ----
            for hg in range(4):
                psC = ps_r.tile([128, 16], F32, tag="psT", name="psC", bufs=2)
                nc.tensor.transpose(psC[:, :10], c_sb[:, hg * 128:(hg + 1) * 128],
                                    id128f_sb[:10, :10])
                nc.vector.tensor_copy(cT[:, hg * 10:(hg + 1) * 10], psC[:, :10])

        # =================== tail: AllGather v, full-batch decoder ===========
        nc.sync.dma_start(vin[:], v_sb[:])
        nc.gpsimd.collective_compute(
            "AllGather", ALU.bypass, replica_groups=rg,
            ins=[vin[:].opt()], outs=[vall[:].opt()])

        vfull = [rt.tile([128, 160], F32, tag=f"vf{bh}", name=f"vf{bh}") for bh in range(2)]
        ecl = [rt.tile([128, 10], F32, tag=f"ecl{bh}", name=f"ecl{bh}") for bh in range(2)]
        sqf = rt.tile([128, 160], F32, tag="sqf", name="sqf")
        cl = rt.tile([128, 10], F32, tag="cl", name="cl")
        psZ = ps_r.tile([1, 16], F32, tag="psA", name="psZ", bufs=1)
        for bh in range(2):
            nc.sync.dma_start(vfull[bh][:],
                              mkap(vall[:], bh * 128 * 160, [[160, 128], [1, 160]]))
            nc.vector.tensor_tensor(sqf[:], vfull[bh][:], vfull[bh][:], op=ALU.mult)
            nc.vector.tensor_reduce(cl[:], sqf[:].rearrange("p (c o) -> p c o", c=10),
                                    axis=AX.X, op=ALU.add)
            nc.scalar.activation(cl[:], cl[:], ACTF.Sqrt)
            nc.scalar.activation(ecl[bh][:], cl[:], ACTF.Exp)
            nc.tensor.matmul(psZ[:, :10], ones128_sb[:], ecl[bh][:],
                             start=(bh == 0), stop=(bh == 1))
        zrow = rt.tile([1, 16], F32, tag="zrow", name="zrow")
        nc.vector.memset(zrow[:], 0)
        nc.vector.tensor_copy(zrow[:, :10], psZ[:, :10])
        nc.vector.reciprocal(zrow[:, :10], zrow[:, :10])
        nc.sync.dma_start(z_in[:], zrow[:])
        zfull = rt.tile([128, 10], F32, tag="zfull", name="zfull")
        nc.gpsimd.dma_start(zfull[:], mkap(z_in[:], 0, [[0, 128], [1, 10]]))

        tpr = rt.tile([128, 10], F32, tag="tpr", name="tpr")
        tmax = rt.tile([128, 1], F32, tag="tmax", name="tmax")
        mask = rt.tile([128, 10], F32, tag="mask", name="mask")
        flat = rt.tile([128, 160], BF, tag="flat", name="flat")
        flatTa = rt.tile([128, 256], BF, tag="flatTa", name="flatTa")
        flatTb = rt.tile([32, 256], BF, tag="flatTb", name="flatTb")
        h1q = [rt.tile([128, 256], BF, tag=f"h1q{i}", name=f"h1q{i}") for i in range(4)]
        for bh in range(2):
            nc.vector.tensor_tensor(tpr[:], ecl[bh][:], zfull[:], op=ALU.mult)
            nc.vector.tensor_reduce(tmax[:], tpr[:], axis=AX.X, op=ALU.max)
            nc.vector.tensor_scalar(mask[:], tpr[:], tmax[:], None, op0=ALU.is_equal)
            mb = mask[:].unsqueeze(2).broadcast_to([128, 10, 16])
            nc.vector.tensor_tensor(flat[:].rearrange("p (c o) -> p c o", c=10),
                                    vfull[bh][:].rearrange("p (c o) -> p c o", c=10),
                                    mb, op=ALU.mult)
            psT1 = ps_r.tile([128, 128], BF, tag="psT", name="psT1", bufs=2)
            nc.tensor.transpose(psT1[:], flat[:, 0:128], id128_sb[:])
            nc.vector.tensor_copy(flatTa[:, bh * 128:(bh + 1) * 128], psT1[:])
            psT2 = ps_r.tile([32, 128], BF, tag="psT", name="psT2", bufs=2)
            nc.tensor.transpose(psT2[:], flat[:, 128:160], id128_sb[:])
            nc.vector.tensor_copy(flatTb[:, bh * 128:(bh + 1) * 128], psT2[:])
        # fc1: h1 = relu(w1 @ flat + b1)   [512, 256]
        for fc in range(4):
            ps1 = ps_r.tile([128, 256], F32, tag="psD", name="ps1", bufs=2)
            nc.tensor.matmul(ps1[:], w1ta_sb[:, fc * 128:(fc + 1) * 128], flatTa[:],
                             start=True, stop=False)
            nc.tensor.matmul(ps1[:], w1tb_sb[:, fc * 128:(fc + 1) * 128], flatTb[:],
                             start=False, stop=True)
            nc.scalar.activation(h1q[fc][:], ps1[:], ACTF.Relu, bias=b1d_sb[fc][:],
                                 scale=1.0)
        # fc2: h2 = relu(w2 @ h1 + b2), quantized to fp8 * S_H2
        for gc in range(8):
            ps2 = ps_r.tile([128, 256], F32, tag="psD", name="ps2", bufs=2)
            for kc in range(4):
                nc.tensor.matmul(ps2[:], w2t_sb[kc][:, gc * 128:(gc + 1) * 128],
                                 h1q[kc][:], start=(kc == 0), stop=(kc == 3))
            nc.scalar.activation(h2q_sb[:, gc, :], ps2[:], ACTF.Relu,
                                 bias=b2s_sb[gc][:], scale=S_H2)

    # =================== final big layer (fp8 DoubleRow, weights resident) ====
    with tc.tile_pool(name="ps_o", bufs=4, space="PSUM") as ps_o, \
         tc.tile_pool(name="osb", bufs=4) as osbp:
        w3v = w3q_sb[:].rearrange("p (w r n j) -> p w r j n", w=NW, r=4, j=2)
        for w in range(NW):
            for bh in range(2):
                pso = ps_o.tile([128, 512], F32, tag="pso", name="pso")
                for pr in range(4):
                    nc.tensor.matmul(pso[:],
                                     h2q_sb[:, 2 * pr:2 * pr + 2, bh * 128:(bh + 1) * 128],
                                     w3v[:, w, pr], start=(pr == 0), stop=False,
                                     perf_mode=PM.DoubleRow)
                nc.tensor.matmul(pso[:], onesrow_sb[:],
                                 b3q_sb[:, w * 512:(w + 1) * 512],
                                 start=False, stop=True)
                ot = osbp.tile([128, 512], BF, tag="ot", name="ot")
                nc.scalar.activation(ot[:], pso[:], ACTF.Sigmoid, scale=1.0 / (S_H2 * S_W3))
                nc.sync.dma_start(out_ext[bh * 128:(bh + 1) * 128,
                                          w * 512:(w + 1) * 512], ot[:])
    w3cm.__exit__(None, None, None)


_NC_CACHE = {}


def _host_prep(inputs):
    data = np.asarray(inputs["data"], np.float32)
    conv1_w = np.asarray(inputs["conv1_w"], np.float32)
    conv1_b = np.asarray(inputs["conv1_b"], np.float32)
    prim_w = np.asarray(inputs["prim_w"], np.float32)
    prim_b = np.asarray(inputs["prim_b"], np.float32)
    W_digit = np.asarray(inputs["W_digit"], np.float32)
    dec_w1 = np.asarray(inputs["dec_w1"], np.float32)
    dec_b1 = np.asarray(inputs["dec_b1"], np.float32)
    dec_w2 = np.asarray(inputs["dec_w2"], np.float32)
    dec_b2 = np.asarray(inputs["dec_b2"], np.float32)
    dec_w3 = np.asarray(inputs["dec_w3"], np.float32)
    dec_b3 = np.asarray(inputs["dec_b3"], np.float32)

    w1c = np.ascontiguousarray(conv1_w[:, 0].transpose(1, 2, 0).reshape(81, 256)).astype(BF16)
    wp2q = np.ascontiguousarray(
        prim_w.transpose(2, 3, 1, 0).reshape(20736, 256) * S_WP).astype(FP8)
    # Wfull2 [co=(cc,cl), (j(h,g,i), cd, o)]; route r = 256h + 16cc + 2cl + g
    Wv = W_digit.reshape(2, 16, 8, 2, 10, 16, 8)  # [h, cc, cl, g, cd, o, i]
    wfull = np.ascontiguousarray(Wv.transpose(1, 2, 0, 3, 6, 4, 5)).reshape(128, 5120).astype(BF16)
    w1t = np.ascontiguousarray(dec_w1.T).astype(BF16)
    w2t = np.ascontiguousarray(dec_w2.T).astype(BF16)
    w3t = np.ascontiguousarray(dec_w3.T)  # [1024, 102400]

    common = dict(
        w1c=w1c, b1s=(conv1_b * S_H).reshape(256, 1),
        bp2=prim_b.reshape(256, 1), wp2q=wp2q, wfull=wfull,
        id128=np.eye(128, dtype=np.float32).astype(BF16),
        id128f=np.eye(128, dtype=np.float32),
        w1t=w1t, b1d=dec_b1.reshape(512, 1),
        w2t=w2t, b2s=(dec_b2 * S_H2).reshape(1024, 1),
        onesrow=np.ones((1, 128), np.float32).astype(BF16),
        ones128=np.ones((128, 1), np.float32),
    )
    in_maps = []
    for c in range(NCORES):
        m = dict(common)
        sw = np.lib.stride_tricks.sliding_window_view(
            data[c * BL:(c + 1) * BL, 0], (9, 9), axis=(1, 2))
        m["pat1h"] = np.ascontiguousarray(
            sw.transpose(3, 4, 1, 2, 0).reshape(81, 4608)).astype(BF16)
        w3c = w3t[:, c * RECL:(c + 1) * RECL] * S_W3   # [1024, 12800]
        m["w3q"] = np.ascontiguousarray(
            w3c.reshape(4, 2, 128, NW, 512).transpose(2, 3, 0, 4, 1).reshape(128, 102400)
        ).astype(FP8)
        m["b3q"] = (dec_b3[c * RECL:(c + 1) * RECL] * (S_H2 * S_W3)).reshape(1, RECL).astype(BF16)
        in_maps.append(m)
    return in_maps


def kernel(**inputs):
    if "nc" not in _NC_CACHE:
        _NC_CACHE["nc"] = build_program()
    nc = _NC_CACHE["nc"]
    in_maps = _host_prep(inputs)
    res = run_bass_kernel_spmd(nc, in_maps, list(range(NCORES)))
    outs = [np.asarray(res.results[c]["out"]).astype(np.float32) for c in range(NCORES)]
    rec = np.concatenate(outs, axis=1)
    return rec.reshape(B, 256, 20, 20)


# revision 22
# speedup vs baseline: 1.5168x; 1.1796x over previous
import sys

sys.path.insert(0, "/opt/trn_rl_repo")

import numpy as np
import ml_dtypes

import concourse.bass as bass
import concourse.mybir as mybir
import concourse.tile as tile
from concourse import bacc
from concourse.bass_utils import run_bass_kernel_spmd

BF16 = ml_dtypes.bfloat16
FP8 = ml_dtypes.float8_e4m3
F32 = mybir.dt.float32
BF = mybir.dt.bfloat16
F8 = mybir.dt.float8e4
ALU = mybir.AluOpType
ACTF = mybir.ActivationFunctionType
AX = mybir.AxisListType
PM = mybir.MatmulPerfMode

NCORES = 8
B = 256
BL = B // NCORES          # 32 local batch
REC = 102400
RECL = REC // NCORES      # 12800 local output cols
NW = RECL // 512          # 25 output windows

S_H = 32.0                # conv1-activation fp8 scale
S_WP = 4096.0             # primary-caps weight fp8 scale
S_H2 = 2.0 ** 23          # h2 fp8 scale
S_W3 = 1024.0             # dec_w3 fp8 scale


def mkap(t, offset, dims):
    """Manual access pattern: dims = [[stride, count], ...] (partition dim first)."""
    return bass.AP(tensor=t.tensor if isinstance(t, bass.AP) else t, offset=offset, ap=dims)


def build_program():
    nc = bacc.Bacc(None, num_devices=NCORES)
    rg = [list(range(NCORES))]

    P = {}
    out_ext = nc.declare_dram_parameter("out", [B, RECL], BF, isOutput=True)
    P["w3q"] = nc.declare_dram_parameter("w3q", [128, 102400], F8, isOutput=False)
    P["w2t"] = nc.declare_dram_parameter("w2t", [512, 1024], BF, isOutput=False)
    P["wfull"] = nc.declare_dram_parameter("wfull", [128, 5120], BF, isOutput=False)
    P["b3q"] = nc.declare_dram_parameter("b3q", [1, RECL], BF, isOutput=False)
    P["w1t"] = nc.declare_dram_parameter("w1t", [160, 512], BF, isOutput=False)
    P["b1d"] = nc.declare_dram_parameter("b1d", [512, 1], F32, isOutput=False)
    P["b2s"] = nc.declare_dram_parameter("b2s", [1024, 1], F32, isOutput=False)
    P["id128"] = nc.declare_dram_parameter("id128", [128, 128], BF, isOutput=False)
    P["id128f"] = nc.declare_dram_parameter("id128f", [128, 128], F32, isOutput=False)
    P["onesrow"] = nc.declare_dram_parameter("onesrow", [1, 128], BF, isOutput=False)
    P["ones128"] = nc.declare_dram_parameter("ones128", [128, 1], F32, isOutput=False)
    P["wp2q"] = nc.declare_dram_parameter("wp2q", [20736, 256], F8, isOutput=False)
    P["bp2"] = nc.declare_dram_parameter("bp2", [256, 1], F32, isOutput=False)
    P["b1s"] = nc.declare_dram_parameter("b1s", [256, 1], F32, isOutput=False)
    P["w1c"] = nc.declare_dram_parameter("w1c", [81, 256], BF, isOutput=False)
    P["pat1h"] = nc.declare_dram_parameter("pat1h", [81, 4608], BF, isOutput=False)

    with tile.TileContext(nc) as tc:
        _body(nc, tc, P, out_ext, rg)
    nc.compile()
    return nc


def _body(nc, tc, P, out_ext, rg):
    es = tc.tile_pool(name="const", bufs=1)
    const = es.__enter__()
    dram_cm = tc.tile_pool(name="dram", bufs=1, space="DRAM")
    dram = dram_cm.__enter__()

    # ---------- DRAM scratch ----------
    warm_in = dram.tile([1, 16], F32, tag="warm_in", name="warm_in")
    warm_out = dram.tile([1, 16], F32, tag="warm_out", name="warm_out")
    xdram = dram.tile([2, 128, 512], BF, tag="xdram", name="xdram")
    warm2_in = dram.tile([40, 128], F32, tag="warm2_in", name="warm2_in")
    warm2_out = dram.tile([40, 128], F32, tag="warm2_out", name="warm2_out")
    ar_in = dram.tile([40, 128], F32, tag="ar_in", name="ar_in")
    ar_out = dram.tile([40, 128], F32, tag="ar_out", name="ar_out")
    c2d = dram.tile([10, 512], BF, tag="c2d", name="c2d")
    z_in = dram.tile([1, 16], F32, tag="z_in", name="z_in")
    z_out = dram.tile([1, 16], F32, tag="z_out", name="z_out")
    vin = dram.tile([BL, 160], F32, tag="vin", name="vin")
    vall = dram.tile([NCORES, BL, 160], F32, tag="vall", name="vall")

    # conv1 im2col patches: most critical load, on the gpsimd ring (less
    # contended by the input-upload traffic at kernel start)
    pat1 = const.tile([81, 4608], BF, tag="pat1", name="pat1")
    nc.gpsimd.dma_start(pat1[:], P["pat1h"][:])

    # ---------- constants to SBUF ----------
    w1c_sb = const.tile([81, 256], BF, tag="w1c", name="w1c")
    nc.gpsimd.dma_start(w1c_sb[:], P["w1c"][:])
    # warmup collectives (absorb first-collective setup cost); queued on
    # gpsimd after the two critical front loads
    zw2 = const.tile([40, 128], F32, tag="zw2", name="zw2")
    nc.vector.memset(zw2[:], 0)
    nc.sync.dma_start(warm2_in[:], zw2[:])
    nc.gpsimd.collective_compute(
        "AllReduce", ALU.add, replica_groups=rg,
        ins=[warm2_in[:].opt()], outs=[warm2_out[:].opt()])
    b1s_sb = [const.tile([128, 1], F32, tag=f"b1s{h}", name=f"b1s{h}") for h in range(2)]
    bp2_sb = [const.tile([128, 1], F32, tag=f"bp2{h}", name=f"bp2{h}") for h in range(2)]
    for h in range(2):
        nc.sync.dma_start(b1s_sb[h][:], P["b1s"][h * 128:(h + 1) * 128, :])
        nc.sync.dma_start(bp2_sb[h][:], P["bp2"][h * 128:(h + 1) * 128, :])
    wfull_sb = const.tile([128, 5120], BF, tag="wfull", name="wfull")
    id128_sb = const.tile([128, 128], BF, tag="id128", name="id128")
    nc.sync.dma_start(id128_sb[:], P["id128"][:])
    id128f_sb = const.tile([128, 128], F32, tag="id128f", name="id128f")
    nc.sync.dma_start(id128f_sb[:], P["id128f"][:])
    w1ta_sb = const.tile([128, 512], BF, tag="w1ta", name="w1ta")
    nc.sync.dma_start(w1ta_sb[:], P["w1t"][0:128, :])
    w1tb_sb = const.tile([32, 512], BF, tag="w1tb", name="w1tb")
    nc.sync.dma_start(w1tb_sb[:], P["w1t"][128:160, :])
    b1d_sb = [const.tile([128, 1], F32, tag=f"b1d{i}", name=f"b1d{i}") for i in range(4)]
    for i in range(4):
        nc.sync.dma_start(b1d_sb[i][:], P["b1d"][i * 128:(i + 1) * 128, :])
    w2t_sb = [const.tile([128, 1024], BF, tag=f"w2t{i}", name=f"w2t{i}") for i in range(4)]
    b2s_sb = [const.tile([128, 1], F32, tag=f"b2s{i}", name=f"b2s{i}") for i in range(8)]
    for i in range(8):
        nc.sync.dma_start(b2s_sb[i][:], P["b2s"][i * 128:(i + 1) * 128, :])
    onesrow_sb = const.tile([1, 128], BF, tag="onesrow", name="onesrow")
    nc.sync.dma_start(onesrow_sb[:], P["onesrow"][:])
    ones128_sb = const.tile([128, 1], F32, tag="ones128", name="ones128")
    nc.sync.dma_start(ones128_sb[:], P["ones128"][:])
    b3q_sb = const.tile([1, RECL], BF, tag="b3q", name="b3q")

    # persistent mid-size tiles
    X = [const.tile([128, 512], BF, tag=f"X{h}", name=f"X{h}") for h in range(2)]
    x2_sb = const.tile([BL, 4096], BF, tag="x2", name="x2")       # [b, (j,co)]
    wc_sb = const.tile([128, 5120], BF, tag="wc", name="wc")      # c-weighted W / prod scratch
    h2q_sb = const.tile([128, 8, 256], F8, tag="h2q", name="h2q")  # [feat%128, kc, b]

    # =================== conv1 + primary caps (fp8 DoubleRow) ===================
    with tc.tile_pool(name="front", bufs=1) as front, \
         tc.tile_pool(name="ps_f", bufs=2, space="PSUM") as ps_f:
        H = front.tile([128, 4608, 2], F8, tag="H", name="H")
        # full primary-caps weight, preloaded (k-pairs in dim1)
        wp2q_sb = front.tile([128, 162, 256], F8, tag="wp2q", name="wp2q")
        for k in range(3):
            nc.sync.dma_start(wp2q_sb[:, 54 * k:54 * (k + 1), :],
                              mkap(P["wp2q"], 54 * k * 32768,
                                   [[256, 128], [32768, 54], [1, 256]]))
        for h in range(2):
            for w in range(9):
                ps = ps_f.tile([128, 512], F32, tag="c1ps", name="c1ps")
                nc.tensor.matmul(ps[:], w1c_sb[:, h * 128:(h + 1) * 128],
                                 pat1[:, w * 512:(w + 1) * 512],
                                 start=True, stop=True)
                nc.scalar.activation(H[:, w * 512:(w + 1) * 512, h], ps[:],
                                     ACTF.Relu, bias=b1s_sb[h][:], scale=S_H)
        U = [front.tile([128, 512], F32, tag=f"U{h}", name=f"U{h}") for h in range(2)]
        psU = [ps_f.tile([128, 512], F32, tag=f"Ups{h}", name=f"Ups{h}", bufs=1) for h in range(2)]
        Hv = H[:].rearrange("p (y x b) c -> p c y x b", y=12, x=12)
        for h in range(2):
            for u in range(81):
                dy, dx = divmod(u, 9)
                rhs = Hv[:, :, dy:dy + 4, dx:dx + 4, :]
                nc.tensor.matmul(psU[h][:],
                                 wp2q_sb[:, 2 * u:2 * u + 2, h * 128:(h + 1) * 128],
                                 rhs, start=(u == 0), stop=(u == 80),
                                 perf_mode=PM.DoubleRow)
        for h in range(2):
            nc.scalar.activation(U[h][:], psU[h][:], ACTF.Identity,
                                 bias=bp2_sb[h][:], scale=1.0 / (S_H * S_WP))
        # big weight loads, delayed so they don't contend with the front stream
        nc.sync.dma_start(wfull_sb[:], P["wfull"][:])
        for i in range(4):
            nc.sync.dma_start(w2t_sb[i][:], P["w2t"][i * 128:(i + 1) * 128, :])
        nc.sync.dma_start(b3q_sb[:], P["b3q"][:])

        # ---- squash -> x (bf16), to DRAM, reload transposed ----
        usq = front.tile([128, 512], F32, tag="usq", name="usq")
        sn = front.tile([128, 64], F32, tag="sn", name="sn")
        g = front.tile([128, 64], F32, tag="g", name="g")
        gt = front.tile([128, 64], F32, tag="gt", name="gt")
        for h in range(2):
            nc.vector.tensor_tensor(usq[:], U[h][:], U[h][:], op=ALU.mult)
            uview = usq[:].rearrange("p (g i b) -> p g b i", g=2, i=8)
            nc.vector.tensor_reduce(sn[:].rearrange("p (g b) -> p g b", g=2),
                                    uview, axis=AX.X, op=ALU.add)
            nc.scalar.activation(gt[:], sn[:], ACTF.Sqrt)
            nc.vector.tensor_scalar_add(g[:], sn[:], 1.0)
            nc.vector.reciprocal(g[:], g[:])
            nc.vector.tensor_tensor(g[:], g[:], gt[:], op=ALU.mult)
            gb = g[:].rearrange("p (g b) -> p g b", g=2).unsqueeze(2).broadcast_to(
                [128, 2, 8, BL])
            nc.vector.tensor_tensor(X[h][:].rearrange("p (g i b) -> p g i b", g=2, i=8),
                                    U[h][:].rearrange("p (g i b) -> p g i b", g=2, i=8),
                                    gb, op=ALU.mult)

    w3cm = tc.tile_pool(name="w3pool", bufs=1)
    w3pool = w3cm.__enter__()
    w3q_sb = w3pool.tile([128, 102400], F8, tag="w3q", name="w3q")
    nc.scalar.dma_start(w3q_sb[:], P["w3q"][:])

    # =================== routing (3 iters, matmul-factored) ===================
    with tc.tile_pool(name="route", bufs=1) as rt, \
         tc.tile_pool(name="ps_r", bufs=1, space="PSUM") as ps_r:
        s_sb = rt.tile([BL, 160], F32, tag="s_sb", name="s_sb")
        sq = rt.tile([BL, 160], F32, tag="sq", name="sq")
        num = rt.tile([BL, 160], F32, tag="num", name="num")
        dn = rt.tile([BL, 160], F32, tag="dn", name="dn")
        v_sb = rt.tile([BL, 160], F32, tag="v_sb", name="v_sb")
        vq = rt.tile([BL, 160], BF, tag="vq", name="vq")
        arin_sb = rt.tile([40, 128], F32, tag="arin_sb", name="arin_sb")
        b_accT = rt.tile([40, 128], F32, tag="b_accT", name="b_accT")
        braw = rt.tile([128, 40], F32, tag="braw", name="braw")
        csf = rt.tile([10, 512], F32, tag="csf", name="csf")
        rmax = rt.tile([10, 1], F32, tag="rmax", name="rmax")
        nbias = rt.tile([10, 1], F32, tag="nbias", name="nbias")
        esum = rt.tile([10, 1], F32, tag="esum", name="esum")
        c_sb = rt.tile([10, 512], F32, tag="c_sb", name="c_sb")
        cT = rt.tile([128, 40], F32, tag="cT", name="cT")

        def xsl(j):
            h, gg, ii = j >> 4, (j >> 3) & 1, j & 7
            return X[h][:].rearrange("p (g i b) -> p g i b", g=2, i=8)[:, gg, ii, :]

        for it in range(3):
            # ---- s[b,cd,o] = sum_{r,i} c.W.x  via 32 accumulated matmuls ----
            psS = ps_r.tile([BL, 160], F32, tag="psS", name="psS")
            if it == 0:
                for j in range(32):
                    nc.tensor.matmul(psS[:], xsl(j),
                                     wfull_sb[:, j * 160:(j + 1) * 160],
                                     start=(j == 0), stop=(j == 31))
                nc.vector.tensor_scalar(s_sb[:], psS[:], 1.0 / 512.0, None, op0=ALU.mult)
            else:
                cvv = cT[:].rearrange("p (hg c) -> p hg c", hg=4).unsqueeze(2)\
                    .unsqueeze(4)
                wfv = wfull_sb[:].rearrange("p (hg i c o) -> p hg i c o", hg=4, i=8, c=10)
                wcv = wc_sb[:].rearrange("p (hg i c o) -> p hg i c o", hg=4, i=8, c=10)
                for seg in range(4):
                    nc.vector.tensor_tensor(
                        wcv[:, seg], wfv[:, seg],
                        cvv[:, seg].broadcast_to([128, 8, 10, 16]), op=ALU.mult)
                    for j in range(8 * seg, 8 * (seg + 1)):
                        nc.tensor.matmul(psS[:], xsl(j),
                                         wc_sb[:, j * 160:(j + 1) * 160],
                                         start=(j == 0), stop=(j == 31))
                nc.vector.tensor_copy(s_sb[:], psS[:])
            # ---- elementwise squash: v = sq*s/((1+sq)*sqrt(sq)) ----
            nc.vector.tensor_tensor(sq[:], s_sb[:], s_sb[:], op=ALU.mult)
            nc.vector.tensor_tensor(num[:], sq[:], s_sb[:], op=ALU.mult)
            nc.vector.tensor_scalar_add(dn[:], sq[:], 1.0)
            nc.scalar.activation(sq[:], sq[:], ACTF.Sqrt)
            nc.vector.tensor_tensor(dn[:], dn[:], sq[:], op=ALU.mult)
            nc.vector.reciprocal(dn[:], dn[:])
            nc.vector.tensor_tensor(v_sb[:], num[:], dn[:], op=ALU.mult)

            if it == 2:
                break
            if it == 0:
                # x2[b, (j,co)] via 32 tensor transposes of X slices
                for j in range(32):
                    psT = ps_r.tile([32, 128], BF, tag="psT", name="psT", bufs=2)
                    nc.tensor.transpose(psT[:], xsl(j), id128_sb[:])
                    nc.vector.tensor_copy(x2_sb[:, j * 128:(j + 1) * 128], psT[:])
            # ---- G[(r,i),(cd,o)] = sum_b x v; prod = G.W fused from psum ----
            nc.vector.tensor_copy(vq[:], v_sb[:])
            for j in range(32):
                psG = ps_r.tile([128, 160], F32, tag="psG", name="psG", bufs=2)
                nc.tensor.matmul(psG[:], x2_sb[:, j * 128:(j + 1) * 128], vq[:],
                                 start=True, stop=True)
                nc.vector.tensor_tensor(wc_sb[:, j * 160:(j + 1) * 160], psG[:],
                                        wfull_sb[:, j * 160:(j + 1) * 160], op=ALU.mult)
            # ---- a_mean[r,cd] = sum_{i,o} prod : one strided XY reduce ----
            prv = wc_sb[:].rearrange("p (h g i c o) -> p h g c i o", h=2, g=2, i=8, c=10, o=16)
            nc.vector.tensor_reduce(
                braw[:].rearrange("p (c h g) -> p h g c", c=10, h=2),
                prv, axis=AX.XY, op=ALU.add)
            # transpose to [(cd,h,g), co] and fold previous b-state (/256 batch,
            # /8 cores so the AllReduce sum is the new b directly)
            psB = ps_r.tile([40, 128], F32, tag="psA", name="psB")
            nc.tensor.transpose(psB[:], braw[:], id128f_sb[:])
            if it == 0:
                nc.vector.tensor_scalar(arin_sb[:], psB[:], 1.0 / 256.0, None, op0=ALU.mult)
            else:
                nc.vector.scalar_tensor_tensor(arin_sb[:], psB[:], 1.0 / 256.0,
                                               b_accT[:], op0=ALU.mult, op1=ALU.add)
            nc.sync.dma_start(ar_in[:], arin_sb[:])
            nc.gpsimd.collective_compute(
                "AllReduce", ALU.add, replica_groups=rg,
                ins=[ar_in[:].opt()], outs=[ar_out[:].opt()])
            # b_accT := b_new/8  (next round each core contributes b/8 so the
            # 8-way sum reconstructs b)
            nc.sync.dma_start(csf[:], mkap(ar_out[:], 0,
                                           [[512, 10], [256, 2], [128, 2], [1, 128]]))
            nc.gpsimd.dma_start(b_accT[:], ar_out[:])
            nc.vector.tensor_scalar(b_accT[:], b_accT[:], 0.125, None, op0=ALU.mult)
            # ---- softmax over routes (free dim) ----
            nc.vector.tensor_reduce(rmax[:], csf[:], axis=AX.X, op=ALU.max)
            nc.scalar.mul(nbias[:], rmax[:], -1.0)
            nc.scalar.activation(c_sb[:], csf[:], ACTF.Exp, bias=nbias[:], scale=1.0)
            nc.vector.tensor_reduce(esum[:], c_sb[:], axis=AX.X, op=ALU.add)
            nc.vector.reciprocal(esum[:], esum[:])
            nc.vector.tensor_scalar_mul(c_sb[:], c_sb[:], esum[:])
            # ---- cT[co, (h,g,cd)] via 4 on-chip transposes ----
            for hg in range(4):
                psC = ps_r.tile([128, 16], F32, tag="psT", name="psC", bufs=2)
                nc.tensor.transpose(psC[:, :10], c_sb[:, hg * 128:(hg + 1) * 128],
                                    id128f_sb[:10, :10])
                nc.vector.tensor_copy(cT[:, hg * 10:(hg + 1) * 10], psC[:, :10])

        # =================== tail: AllGather v, full-batch decoder ===========
        nc.sync.dma_start(vin[:], v_sb[:])
        nc.gpsimd.collective_compute(
            "AllGather", ALU.bypass, replica_groups=rg,
            ins=[vin[:].opt()], outs=[vall[:].opt()])

        vfull = [rt.tile([128, 160], F32, tag=f"vf{bh}", name=f"vf{bh}") for bh in range(2)]
        ecl = [rt.tile([128, 10], F32, tag=f"ecl{bh}", name=f"ecl{bh}") for bh in range(2)]
        sqf = rt.tile([128, 160], F32, tag="sqf", name="sqf")
        cl = rt.tile([128, 10], F32, tag="cl", name="cl")
        psZ = ps_r.tile([1, 16], F32, tag="psA", name="psZ", bufs=1)
        for bh in range(2):
            nc.sync.dma_start(vfull[bh][:],
                              mkap(vall[:], bh * 128 * 160, [[160, 128], [1, 160]]))
            nc.vector.tensor_tensor(sqf[:], vfull[bh][:], vfull[bh][:], op=ALU.mult)
            nc.vector.tensor_reduce(cl[:], sqf[:].rearrange("p (c o) -> p c o", c=10),
                                    axis=AX.X, op=ALU.add)
            nc.scalar.activation(cl[:], cl[:], ACTF.Sqrt)
            nc.scalar.activation(ecl[bh][:], cl[:], ACTF.Exp)
            nc.tensor.matmul(psZ[:, :10], ones128_sb[:], ecl[bh][:],
                             start=(bh == 0), stop=(bh == 1))
        zrow = rt.tile([1, 16], F32, tag="zrow", name="zrow")
        nc.vector.memset(zrow[:], 0)
        nc.vector.tensor_copy(zrow[:, :10], psZ[:, :10])
        nc.vector.reciprocal(zrow[:, :10], zrow[:, :10])
        nc.sync.dma_start(z_in[:], zrow[:])
        zfull = rt.tile([128, 10], F32, tag="zfull", name="zfull")
        nc.gpsimd.dma_start(zfull[:], mkap(z_in[:], 0, [[0, 128], [1, 10]]))

        tpr = rt.tile([128, 10], F32, tag="tpr", name="tpr")
        tmax = rt.tile([128, 1], F32, tag="tmax", name="tmax")
        mask = rt.tile([128, 10], F32, tag="mask", name="mask")
        flat = rt.tile([128, 160], BF, tag="flat", name="flat")
        flatTa = rt.tile([128, 256], BF, tag="flatTa", name="flatTa")
        flatTb = rt.tile([32, 256], BF, tag="flatTb", name="flatTb")
        h1q = [rt.tile([128, 256], BF, tag=f"h1q{i}", name=f"h1q{i}") for i in range(4)]
        for bh in range(2):
            nc.vector.tensor_tensor(tpr[:], ecl[bh][:], zfull[:], op=ALU.mult)
            nc.vector.tensor_reduce(tmax[:], tpr[:], axis=AX.X, op=ALU.max)
            nc.vector.tensor_scalar(mask[:], tpr[:], tmax[:], None, op0=ALU.is_equal)
            mb = mask[:].unsqueeze(2).broadcast_to([128, 10, 16])
            nc.vector.tensor_tensor(flat[:].rearrange("p (c o) -> p c o", c=10),
                                    vfull[bh][:].rearrange("p (c o) -> p c o", c=10),
                                    mb, op=ALU.mult)
            psT1 = ps_r.tile([128, 128], BF, tag="psT", name="psT1", bufs=2)
            nc.tensor.transpose(psT1[:], flat[:, 0:128], id128_sb[:])
            nc.vector.tensor_copy(flatTa[:, bh * 128:(bh + 1) * 128], psT1[:])
            psT2 = ps_r.tile([32, 128], BF, tag="psT", name="psT2", bufs=2)
            nc.tensor.transpose(psT2[:], flat[:, 128:160], id128_sb[:])
            nc.vector.tensor_copy(flatTb[:, bh * 128:(bh + 1) * 128], psT2[:])
        # fc1: h1 = relu(w1 @ flat + b1)   [512, 256]
        for fc in range(4):
            ps1 = ps_r.tile([128, 256], F32, tag="psD", name="ps1", bufs=2)
            nc.tensor.matmul(ps1[:], w1ta_sb[:, fc * 128:(fc + 1) * 128], flatTa[:],
                             start=True, stop=False)
            nc.tensor.matmul(ps1[:], w1tb_sb[:, fc * 128:(fc + 1) * 128], flatTb[:],
                             start=False, stop=True)
            nc.scalar.activation(h1q[fc][:], ps1[:], ACTF.Relu, bias=b1d_sb[fc][:],
                                 scale=1.0)
        # fc2: h2 = relu(w2 @ h1 + b2), quantized to fp8 * S_H2
        for gc in range(8):
            ps2 = ps_r.tile([128, 256], F32, tag="psD", name="ps2", bufs=2)
            for kc in range(4):
                nc.tensor.matmul(ps2[:], w2t_sb[kc][:, gc * 128:(gc + 1) * 128],
                                 h1q[kc][:], start=(kc == 0), stop=(kc == 3))
            nc.scalar.activation(h2q_sb[:, gc, :], ps2[:], ACTF.Relu,
                                 bias=b2s_sb[gc][:], scale=S_H2)

    # =================== final big layer (fp8 DoubleRow, weights resident) ====
    with tc.tile_pool(name="ps_o", bufs=4, space="PSUM") as ps_o, \
         tc.tile_pool(name="osb", bufs=4) as osbp:
        w3v = w3q_sb[:].rearrange("p (w r n j) -> p w r j n", w=NW, r=4, j=2)
        for w in range(NW):
            for bh in range(2):
                pso = ps_o.tile([128, 512], F32, tag="pso", name="pso")
                for pr in range(4):
                    nc.tensor.matmul(pso[:],
                                     h2q_sb[:, 2 * pr:2 * pr + 2, bh * 128:(bh + 1) * 128],
                                     w3v[:, w, pr], start=(pr == 0), stop=False,
                                     perf_mode=PM.DoubleRow)
                nc.tensor.matmul(pso[:], onesrow_sb[:],
                                 b3q_sb[:, w * 512:(w + 1) * 512],
                                 start=False, stop=True)
                ot = osbp.tile([128, 512], BF, tag="ot", name="ot")
                nc.scalar.activation(ot[:], pso[:], ACTF.Sigmoid, scale=1.0 / (S_H2 * S_W3))
                nc.sync.dma_start(out_ext[bh * 128:(bh + 1) * 128,
                                          w * 512:(w + 1) * 512], ot[:])
    w3cm.__exit__(None, None, None)


_NC_CACHE = {}


def _host_prep(inputs):
    data = np.asarray(inputs["data"], np.float32)
    conv1_w = np.asarray(inputs["conv1_w"], np.float32)
    conv1_b = np.asarray(inputs["conv1_b"], np.float32)
    prim_w = np.asarray(inputs["prim_w"], np.float32)
    prim_b = np.asarray(inputs["prim_b"], np.float32)
    W_digit = np.asarray(inputs["W_digit"], np.float32)
    dec_w1 = np.asarray(inputs["dec_w1"], np.float32)
    dec_b1 = np.asarray(inputs["dec_b1"], np.float32)
    dec_w2 = np.asarray(inputs["dec_w2"], np.float32)
    dec_b2 = np.asarray(inputs["dec_b2"], np.float32)
    dec_w3 = np.asarray(inputs["dec_w3"], np.float32)
    dec_b3 = np.asarray(inputs["dec_b3"], np.float32)

    w1c = np.ascontiguousarray(conv1_w[:, 0].transpose(1, 2, 0).reshape(81, 256)).astype(BF16)
    wp2q = np.ascontiguousarray(
        prim_w.transpose(2, 3, 1, 0).reshape(20736, 256) * S_WP).astype(FP8)
    # Wfull2 [co=(cc,cl), (j(h,g,i), cd, o)]; route r = 256h + 16cc + 2cl + g
    Wv = W_digit.reshape(2, 16, 8, 2, 10, 16, 8)  # [h, cc, cl, g, cd, o, i]
    wfull = np.ascontiguousarray(Wv.transpose(1, 2, 0, 3, 6, 4, 5)).reshape(128, 5120).astype(BF16)
    w1t = np.ascontiguousarray(dec_w1.T).astype(BF16)
    w2t = np.ascontiguousarray(dec_w2.T).astype(BF16)
    w3t = np.ascontiguousarray(dec_w3.T)  # [1024, 102400]

    common = dict(
        w1c=w1c, b1s=(conv1_b * S_H).reshape(256, 1),
        bp2=prim_b.reshape(256, 1), wp2q=wp2q, wfull=wfull,
        id128=np.eye(128, dtype=np.float32).astype(BF16),
        id128f=np.eye(128, dtype=np.float32),
        w1t=w1t, b1d=dec_b1.reshape(512, 1),
        w2t=w2t, b2s=(dec_b2 * S_H2).reshape(1024, 1),
        onesrow=np.ones((1, 128), np.float32).astype(BF16),
        ones128=np.ones((128, 1), np.float32),
    )
    in_maps = []
    for c in range(NCORES):
        m = dict(common)
        sw = np.lib.stride_tricks.sliding_window_view(
            data[c * BL:(c + 1) * BL, 0], (9, 9), axis=(1, 2))
        m["pat1h"] = np.ascontiguousarray(
            sw.transpose(3, 4, 1, 2, 0).reshape(81, 4608)).astype(BF16)
        w3c = w3t[:, c * RECL:(c + 1) * RECL] * S_W3   # [1024, 12800]
        m["w3q"] = np.ascontiguousarray(
            w3c.reshape(4, 2, 128, NW, 512).transpose(2, 3, 0, 4, 1).reshape(128, 102400)
        ).astype(FP8)
        m["b3q"] = (dec_b3[c * RECL:(c + 1) * RECL] * (S_H2 * S_W3)).reshape(1, RECL).astype(BF16)
        in_maps.append(m)
    return in_maps


def kernel(**inputs):
    if "nc" not in _NC_CACHE:
        _NC_CACHE["nc"] = build_program()
    nc = _NC_CACHE["nc"]
    in_maps = _host_prep(inputs)
    res = run_bass_kernel_spmd(nc, in_maps, list(range(NCORES)))
    outs = [np.asarray(res.results[c]["out"]).astype(np.float32) for c in range(NCORES)]
    rec = np.concatenate(outs, axis=1)
    return rec.reshape(B, 256, 20, 20)


# revision 24
# speedup vs baseline: 1.5971x; 1.0530x over previous
import sys

sys.path.insert(0, "/opt/trn_rl_repo")

import numpy as np
import ml_dtypes

import concourse.bass as bass
import concourse.mybir as mybir
import concourse.tile as tile
from concourse import bacc
from concourse.bass_utils import run_bass_kernel_spmd

BF16 = ml_dtypes.bfloat16
FP8 = ml_dtypes.float8_e4m3
F32 = mybir.dt.float32
BF = mybir.dt.bfloat16
F8 = mybir.dt.float8e4
ALU = mybir.AluOpType
ACTF = mybir.ActivationFunctionType
AX = mybir.AxisListType
PM = mybir.MatmulPerfMode

NCORES = 8
B = 256
BL = B // NCORES          # 32 local batch
REC = 102400
RECL = REC // NCORES      # 12800 local output cols
NW = RECL // 512          # 25 output windows

S_H = 32.0                # conv1-activation fp8 scale
S_WP = 4096.0             # primary-caps weight fp8 scale
S_H2 = 2.0 ** 23          # h2 fp8 scale
S_W3 = 1024.0             # dec_w3 fp8 scale


def mkap(t, offset, dims):
    """Manual access pattern: dims = [[stride, count], ...] (partition dim first)."""
    return bass.AP(tensor=t.tensor if isinstance(t, bass.AP) else t, offset=offset, ap=dims)


def build_program():
    nc = bacc.Bacc(None, num_devices=NCORES)
    rg = [list(range(NCORES))]

    P = {}
    out_ext = nc.declare_dram_parameter("out", [B, RECL], BF, isOutput=True)
    P["w3q"] = nc.declare_dram_parameter("w3q", [128, 102400], F8, isOutput=False)
    P["w2t"] = nc.declare_dram_parameter("w2t", [512, 1024], BF, isOutput=False)
    P["wfull"] = nc.declare_dram_parameter("wfull", [128, 5120], BF, isOutput=False)
    P["b3q"] = nc.declare_dram_parameter("b3q", [1, RECL], BF, isOutput=False)
    P["w1t"] = nc.declare_dram_parameter("w1t", [160, 512], BF, isOutput=False)
    P["b1d"] = nc.declare_dram_parameter("b1d", [512, 1], F32, isOutput=False)
    P["b2s"] = nc.declare_dram_parameter("b2s", [1024, 1], F32, isOutput=False)
    P["id128"] = nc.declare_dram_parameter("id128", [128, 128], BF, isOutput=False)
    P["id128f"] = nc.declare_dram_parameter("id128f", [128, 128], F32, isOutput=False)
    P["onesrow"] = nc.declare_dram_parameter("onesrow", [1, 128], BF, isOutput=False)
    P["ones128"] = nc.declare_dram_parameter("ones128", [128, 1], F32, isOutput=False)
    P["wp2q"] = nc.declare_dram_parameter("wp2q", [20736, 256], F8, isOutput=False)
    P["bp2"] = nc.declare_dram_parameter("bp2", [256, 1], F32, isOutput=False)
    P["b1s"] = nc.declare_dram_parameter("b1s", [256, 1], F32, isOutput=False)
    P["w1c"] = nc.declare_dram_parameter("w1c", [81, 256], BF, isOutput=False)
    P["pat1h"] = nc.declare_dram_parameter("pat1h", [81, 4608], BF, isOutput=False)

    with tile.TileContext(nc) as tc:
        _body(nc, tc, P, out_ext, rg)
    nc.compile()
    return nc


def _body(nc, tc, P, out_ext, rg):
    es = tc.tile_pool(name="const", bufs=1)
    const = es.__enter__()
    dram_cm = tc.tile_pool(name="dram", bufs=1, space="DRAM")
    dram = dram_cm.__enter__()

    # ---------- DRAM scratch ----------
    warm_in = dram.tile([1, 16], F32, tag="warm_in", name="warm_in")
    warm_out = dram.tile([1, 16], F32, tag="warm_out", name="warm_out")
    xdram = dram.tile([2, 128, 512], BF, tag="xdram", name="xdram")
    warm2_in = dram.tile([40, 128], F32, tag="warm2_in", name="warm2_in")
    warm2_out = dram.tile([40, 128], F32, tag="warm2_out", name="warm2_out")
    ar_in = dram.tile([40, 128], F32, tag="ar_in", name="ar_in")
    ar_out = dram.tile([40, 128], F32, tag="ar_out", name="ar_out")
    c2d = dram.tile([10, 512], BF, tag="c2d", name="c2d")
    z_in = dram.tile([1, 16], F32, tag="z_in", name="z_in")
    z_out = dram.tile([1, 16], F32, tag="z_out", name="z_out")
    vin = dram.tile([BL, 160], F32, tag="vin", name="vin")
    vall = dram.tile([NCORES, BL, 160], F32, tag="vall", name="vall")

    # conv1 im2col patches: most critical load, on the gpsimd ring (less
    # contended by the input-upload traffic at kernel start)
    pat1 = const.tile([81, 4608], BF, tag="pat1", name="pat1")
    nc.gpsimd.dma_start(pat1[:], P["pat1h"][:])

    # ---------- constants to SBUF ----------
    w1c_sb = const.tile([81, 256], BF, tag="w1c", name="w1c")
    nc.gpsimd.dma_start(w1c_sb[:], P["w1c"][:])
    # warmup collectives (absorb first-collective setup cost); queued on
    # gpsimd after the two critical front loads
    zw2 = const.tile([40, 128], F32, tag="zw2", name="zw2")
    nc.vector.memset(zw2[:], 0)
    nc.sync.dma_start(warm2_in[:], zw2[:])
    nc.gpsimd.collective_compute(
        "AllReduce", ALU.add, replica_groups=rg,
        ins=[warm2_in[:].opt()], outs=[warm2_out[:].opt()])
    b1s_sb = [const.tile([128, 1], F32, tag=f"b1s{h}", name=f"b1s{h}") for h in range(2)]
    bp2_sb = [const.tile([128, 1], F32, tag=f"bp2{h}", name=f"bp2{h}") for h in range(2)]
    for h in range(2):
        nc.sync.dma_start(b1s_sb[h][:], P["b1s"][h * 128:(h + 1) * 128, :])
        nc.sync.dma_start(bp2_sb[h][:], P["bp2"][h * 128:(h + 1) * 128, :])
    wfull_sb = const.tile([128, 5120], BF, tag="wfull", name="wfull")
    id128_sb = const.tile([128, 128], BF, tag="id128", name="id128")
    nc.sync.dma_start(id128_sb[:], P["id128"][:])
    id128f_sb = const.tile([128, 128], F32, tag="id128f", name="id128f")
    nc.sync.dma_start(id128f_sb[:], P["id128f"][:])
    w1ta_sb = const.tile([128, 512], BF, tag="w1ta", name="w1ta")
    nc.sync.dma_start(w1ta_sb[:], P["w1t"][0:128, :])
    w1tb_sb = const.tile([32, 512], BF, tag="w1tb", name="w1tb")
    nc.sync.dma_start(w1tb_sb[:], P["w1t"][128:160, :])
    b1d_sb = [const.tile([128, 1], F32, tag=f"b1d{i}", name=f"b1d{i}") for i in range(4)]
    for i in range(4):
        nc.sync.dma_start(b1d_sb[i][:], P["b1d"][i * 128:(i + 1) * 128, :])
    w2t_sb = [const.tile([128, 1024], BF, tag=f"w2t{i}", name=f"w2t{i}") for i in range(4)]
    b2s_sb = [const.tile([128, 1], F32, tag=f"b2s{i}", name=f"b2s{i}") for i in range(8)]
    for i in range(8):
        nc.sync.dma_start(b2s_sb[i][:], P["b2s"][i * 128:(i + 1) * 128, :])
    onesrow_sb = const.tile([1, 128], BF, tag="onesrow", name="onesrow")
    nc.sync.dma_start(onesrow_sb[:], P["onesrow"][:])
    ones128_sb = const.tile([128, 1], F32, tag="ones128", name="ones128")
    nc.sync.dma_start(ones128_sb[:], P["ones128"][:])

    # persistent mid-size tiles
    X = [const.tile([128, 512], BF, tag=f"X{h}", name=f"X{h}") for h in range(2)]
    x2_sb = const.tile([BL, 4096], BF, tag="x2", name="x2")       # [b, (j,co)]
    wc_sb = const.tile([128, 5120], BF, tag="wc", name="wc")      # c-weighted W / prod scratch
    h2q_sb = const.tile([128, 8, 256], F8, tag="h2q", name="h2q")  # [feat%128, kc, b]

    # =================== conv1 + primary caps (fp8 DoubleRow) ===================
    with tc.tile_pool(name="front", bufs=1) as front, \
         tc.tile_pool(name="ps_f", bufs=2, space="PSUM") as ps_f:
        H = front.tile([128, 4608, 2], F8, tag="H", name="H")
        # full primary-caps weight, preloaded (k-pairs in dim1)
        wp2q_sb = front.tile([128, 162, 256], F8, tag="wp2q", name="wp2q")
        for k in range(3):
            nc.sync.dma_start(wp2q_sb[:, 54 * k:54 * (k + 1), :],
                              mkap(P["wp2q"], 54 * k * 32768,
                                   [[256, 128], [32768, 54], [1, 256]]))
        for h in range(2):
            for w in range(9):
                ps = ps_f.tile([128, 512], F32, tag="c1ps", name="c1ps")
                nc.tensor.matmul(ps[:], w1c_sb[:, h * 128:(h + 1) * 128],
                                 pat1[:, w * 512:(w + 1) * 512],
                                 start=True, stop=True)
                nc.scalar.activation(H[:, w * 512:(w + 1) * 512, h], ps[:],
                                     ACTF.Relu, bias=b1s_sb[h][:], scale=S_H)
        U = [front.tile([128, 512], F32, tag=f"U{h}", name=f"U{h}") for h in range(2)]
        psU = [ps_f.tile([128, 512], F32, tag=f"Ups{h}", name=f"Ups{h}", bufs=1) for h in range(2)]
        Hv = H[:].rearrange("p (y x b) c -> p c y x b", y=12, x=12)
        for h in range(2):
            for u in range(81):
                dy, dx = divmod(u, 9)
                rhs = Hv[:, :, dy:dy + 4, dx:dx + 4, :]
                nc.tensor.matmul(psU[h][:],
                                 wp2q_sb[:, 2 * u:2 * u + 2, h * 128:(h + 1) * 128],
                                 rhs, start=(u == 0), stop=(u == 80),
                                 perf_mode=PM.DoubleRow)
        for h in range(2):
            nc.scalar.activation(U[h][:], psU[h][:], ACTF.Identity,
                                 bias=bp2_sb[h][:], scale=1.0 / (S_H * S_WP))
        # big weight loads, delayed so they don't contend with the front stream
        nc.sync.dma_start(wfull_sb[:], P["wfull"][:])
        for i in range(4):
            nc.sync.dma_start(w2t_sb[i][:], P["w2t"][i * 128:(i + 1) * 128, :])

        # ---- squash -> x (bf16), to DRAM, reload transposed ----
        usq = front.tile([128, 512], F32, tag="usq", name="usq")
        sn = front.tile([128, 64], F32, tag="sn", name="sn")
        g = front.tile([128, 64], F32, tag="g", name="g")
        gt = front.tile([128, 64], F32, tag="gt", name="gt")
        for h in range(2):
            nc.vector.tensor_tensor(usq[:], U[h][:], U[h][:], op=ALU.mult)
            uview = usq[:].rearrange("p (g i b) -> p g b i", g=2, i=8)
            nc.vector.tensor_reduce(sn[:].rearrange("p (g b) -> p g b", g=2),
                                    uview, axis=AX.X, op=ALU.add)
            nc.scalar.activation(gt[:], sn[:], ACTF.Sqrt)
            nc.vector.tensor_scalar_add(g[:], sn[:], 1.0)
            nc.vector.reciprocal(g[:], g[:])
            nc.vector.tensor_tensor(g[:], g[:], gt[:], op=ALU.mult)
            gb = g[:].rearrange("p (g b) -> p g b", g=2).unsqueeze(2).broadcast_to(
                [128, 2, 8, BL])
            nc.vector.tensor_tensor(X[h][:].rearrange("p (g i b) -> p g i b", g=2, i=8),
                                    U[h][:].rearrange("p (g i b) -> p g i b", g=2, i=8),
                                    gb, op=ALU.mult)

    w3cm = tc.tile_pool(name="w3pool", bufs=1)
    w3pool = w3cm.__enter__()
    w3q_sb = w3pool.tile([128, 102400], F8, tag="w3q", name="w3q")
    nc.scalar.dma_start(w3q_sb[:], P["w3q"][:])
    b3rep_sb = w3pool.tile([128, RECL], BF, tag="b3rep", name="b3rep")
    nc.gpsimd.dma_start(b3rep_sb[:], mkap(P["b3q"], 0, [[0, 128], [1, RECL]]))

    # =================== routing (3 iters, matmul-factored) ===================
    with tc.tile_pool(name="route", bufs=1) as rt, \
         tc.tile_pool(name="ps_r", bufs=1, space="PSUM") as ps_r:
        s_sb = rt.tile([BL, 160], F32, tag="s_sb", name="s_sb")
        sq = rt.tile([BL, 160], F32, tag="sq", name="sq")
        num = rt.tile([BL, 160], F32, tag="num", name="num")
        dn = rt.tile([BL, 160], F32, tag="dn", name="dn")
        v_sb = rt.tile([BL, 160], F32, tag="v_sb", name="v_sb")
        vq = rt.tile([BL, 160], BF, tag="vq", name="vq")
        arin_sb = rt.tile([40, 128], F32, tag="arin_sb", name="arin_sb")
        b_accT = rt.tile([40, 128], F32, tag="b_accT", name="b_accT")
        braw = rt.tile([128, 40], F32, tag="braw", name="braw")
        csf = rt.tile([10, 512], F32, tag="csf", name="csf")
        rmax = rt.tile([10, 1], F32, tag="rmax", name="rmax")
        nbias = rt.tile([10, 1], F32, tag="nbias", name="nbias")
        esum = rt.tile([10, 1], F32, tag="esum", name="esum")
        c_sb = rt.tile([10, 512], F32, tag="c_sb", name="c_sb")
        cT = rt.tile([128, 40], F32, tag="cT", name="cT")

        def xsl(j):
            h, gg, ii = j >> 4, (j >> 3) & 1, j & 7
            return X[h][:].rearrange("p (g i b) -> p g i b", g=2, i=8)[:, gg, ii, :]

        for it in range(3):
            # ---- s[b,cd,o] = sum_{r,i} c.W.x  via 32 accumulated matmuls ----
            psS = ps_r.tile([BL, 160], F32, tag="psS", name="psS")
            if it == 0:
                for j in range(32):
                    nc.tensor.matmul(psS[:], xsl(j),
                                     wfull_sb[:, j * 160:(j + 1) * 160],
                                     start=(j == 0), stop=(j == 31))
                nc.vector.tensor_scalar(s_sb[:], psS[:], 1.0 / 512.0, None, op0=ALU.mult)
            else:
                cvv = cT[:].rearrange("p (hg c) -> p hg c", hg=4).unsqueeze(2)\
                    .unsqueeze(4)
                wfv = wfull_sb[:].rearrange("p (hg i c o) -> p hg i c o", hg=4, i=8, c=10)
                wcv = wc_sb[:].rearrange("p (hg i c o) -> p hg i c o", hg=4, i=8, c=10)
                for seg in range(4):
                    nc.vector.tensor_tensor(
                        wcv[:, seg], wfv[:, seg],
                        cvv[:, seg].broadcast_to([128, 8, 10, 16]), op=ALU.mult)
                    for j in range(8 * seg, 8 * (seg + 1)):
                        nc.tensor.matmul(psS[:], xsl(j),
                                         wc_sb[:, j * 160:(j + 1) * 160],
                                         start=(j == 0), stop=(j == 31))
                nc.vector.tensor_copy(s_sb[:], psS[:])
            # ---- elementwise squash: v = sq*s/((1+sq)*sqrt(sq)) ----
            nc.vector.tensor_tensor(sq[:], s_sb[:], s_sb[:], op=ALU.mult)
            nc.vector.tensor_tensor(num[:], sq[:], s_sb[:], op=ALU.mult)
            nc.vector.tensor_scalar_add(dn[:], sq[:], 1.0)
            nc.scalar.activation(sq[:], sq[:], ACTF.Sqrt)
            nc.vector.tensor_tensor(dn[:], dn[:], sq[:], op=ALU.mult)
            nc.vector.reciprocal(dn[:], dn[:])
            nc.vector.tensor_tensor(v_sb[:], num[:], dn[:], op=ALU.mult)

            if it == 2:
                break
            if it == 0:
                # x2[b, (j,co)] via 32 tensor transposes of X slices
                for j in range(32):
                    psT = ps_r.tile([32, 128], BF, tag="psT", name="psT", bufs=2)
                    nc.tensor.transpose(psT[:], xsl(j), id128_sb[:])
                    nc.vector.tensor_copy(x2_sb[:, j * 128:(j + 1) * 128], psT[:])
            # ---- G[(r,i),(cd,o)] = sum_b x v; prod = G.W fused from psum ----
            nc.vector.tensor_copy(vq[:], v_sb[:])
            for j in range(32):
                psG = ps_r.tile([128, 160], F32, tag="psG", name="psG", bufs=2)
                nc.tensor.matmul(psG[:], x2_sb[:, j * 128:(j + 1) * 128], vq[:],
                                 start=True, stop=True)
                nc.vector.tensor_tensor(wc_sb[:, j * 160:(j + 1) * 160], psG[:],
                                        wfull_sb[:, j * 160:(j + 1) * 160], op=ALU.mult)
            # ---- a_mean[r,cd] = sum_{i,o} prod : one strided XY reduce ----
            prv = wc_sb[:].rearrange("p (h g i c o) -> p h g c i o", h=2, g=2, i=8, c=10, o=16)
            nc.vector.tensor_reduce(
                braw[:].rearrange("p (c h g) -> p h g c", c=10, h=2),
                prv, axis=AX.XY, op=ALU.add)
            # transpose to [(cd,h,g), co] and fold previous b-state (/256 batch,
            # /8 cores so the AllReduce sum is the new b directly)
            psB = ps_r.tile([40, 128], F32, tag="psA", name="psB")
            nc.tensor.transpose(psB[:], braw[:], id128f_sb[:])
            if it == 0:
                nc.vector.tensor_scalar(arin_sb[:], psB[:], 1.0 / 256.0, None, op0=ALU.mult)
            else:
                nc.vector.scalar_tensor_tensor(arin_sb[:], psB[:], 1.0 / 256.0,
                                               b_accT[:], op0=ALU.mult, op1=ALU.add)
            nc.sync.dma_start(ar_in[:], arin_sb[:])
            nc.gpsimd.collective_compute(
                "AllReduce", ALU.add, replica_groups=rg,
                ins=[ar_in[:].opt()], outs=[ar_out[:].opt()])
            # b_accT := b_new/8  (next round each core contributes b/8 so the
            # 8-way sum reconstructs b)
            nc.sync.dma_start(csf[:], mkap(ar_out[:], 0,
                                           [[512, 10], [256, 2], [128, 2], [1, 128]]))
            nc.gpsimd.dma_start(b_accT[:], ar_out[:])
            nc.vector.tensor_scalar(b_accT[:], b_accT[:], 0.125, None, op0=ALU.mult)
            # ---- softmax over routes (free dim) ----
            nc.vector.tensor_reduce(rmax[:], csf[:], axis=AX.X, op=ALU.max)
            nc.scalar.mul(nbias[:], rmax[:], -1.0)
            nc.scalar.activation(c_sb[:], csf[:], ACTF.Exp, bias=nbias[:], scale=1.0)
            nc.vector.tensor_reduce(esum[:], c_sb[:], axis=AX.X, op=ALU.add)
            nc.vector.reciprocal(esum[:], esum[:])
            nc.vector.tensor_scalar_mul(c_sb[:], c_sb[:], esum[:])
            # ---- cT[co, (h,g,cd)] via 4 on-chip transposes ----
            for hg in range(4):
                psC = ps_r.tile([128, 16], F32, tag="psT", name="psC", bufs=2)
                nc.tensor.transpose(psC[:, :10], c_sb[:, hg * 128:(hg + 1) * 128],
                                    id128f_sb[:10, :10])
                nc.vector.tensor_copy(cT[:, hg * 10:(hg + 1) * 10], psC[:, :10])

        # =================== tail: AllGather v, full-batch decoder ===========
        nc.sync.dma_start(vin[:], v_sb[:])
        nc.gpsimd.collective_compute(
            "AllGather", ALU.bypass, replica_groups=rg,
            ins=[vin[:].opt()], outs=[vall[:].opt()])

        vfull = [rt.tile([128, 160], F32, tag=f"vf{bh}", name=f"vf{bh}") for bh in range(2)]
        ecl = [rt.tile([128, 10], F32, tag=f"ecl{bh}", name=f"ecl{bh}") for bh in range(2)]
        sqf = rt.tile([128, 160], F32, tag="sqf", name="sqf")
        cl = rt.tile([128, 10], F32, tag="cl", name="cl")
        psZ = ps_r.tile([1, 16], F32, tag="psA", name="psZ", bufs=1)
        for bh in range(2):
            nc.sync.dma_start(vfull[bh][:],
                              mkap(vall[:], bh * 128 * 160, [[160, 128], [1, 160]]))
            nc.vector.tensor_tensor(sqf[:], vfull[bh][:], vfull[bh][:], op=ALU.mult)
            nc.vector.tensor_reduce(cl[:], sqf[:].rearrange("p (c o) -> p c o", c=10),
                                    axis=AX.X, op=ALU.add)
            nc.scalar.activation(cl[:], cl[:], ACTF.Sqrt)
            nc.scalar.activation(ecl[bh][:], cl[:], ACTF.Exp)
            nc.tensor.matmul(psZ[:, :10], ones128_sb[:], ecl[bh][:],
                             start=(bh == 0), stop=(bh == 1))
        zrow = rt.tile([1, 16], F32, tag="zrow", name="zrow")
        nc.vector.memset(zrow[:], 0)
        nc.vector.tensor_copy(zrow[:, :10], psZ[:, :10])
        nc.vector.reciprocal(zrow[:, :10], zrow[:, :10])
        nc.sync.dma_start(z_in[:], zrow[:])
        zfull = rt.tile([128, 10], F32, tag="zfull", name="zfull")
        nc.gpsimd.dma_start(zfull[:], mkap(z_in[:], 0, [[0, 128], [1, 10]]))

        tpr = rt.tile([128, 10], F32, tag="tpr", name="tpr")
        tmax = rt.tile([128, 1], F32, tag="tmax", name="tmax")
        mask = rt.tile([128, 10], F32, tag="mask", name="mask")
        flat = rt.tile([128, 160], BF, tag="flat", name="flat")
        flatTa = rt.tile([128, 256], BF, tag="flatTa", name="flatTa")
        flatTb = rt.tile([32, 256], BF, tag="flatTb", name="flatTb")
        h1q = [rt.tile([128, 256], BF, tag=f"h1q{i}", name=f"h1q{i}") for i in range(4)]
        for bh in range(2):
            nc.vector.tensor_tensor(tpr[:], ecl[bh][:], zfull[:], op=ALU.mult)
            nc.vector.tensor_reduce(tmax[:], tpr[:], axis=AX.X, op=ALU.max)
            nc.vector.tensor_scalar(mask[:], tpr[:], tmax[:], None, op0=ALU.is_equal)
            mb = mask[:].unsqueeze(2).broadcast_to([128, 10, 16])
            nc.vector.tensor_tensor(flat[:].rearrange("p (c o) -> p c o", c=10),
                                    vfull[bh][:].rearrange("p (c o) -> p c o", c=10),
                                    mb, op=ALU.mult)
            psT1 = ps_r.tile([128, 128], BF, tag="psT", name="psT1", bufs=2)
            nc.tensor.transpose(psT1[:], flat[:, 0:128], id128_sb[:])
            nc.vector.tensor_copy(flatTa[:, bh * 128:(bh + 1) * 128], psT1[:])
            psT2 = ps_r.tile([32, 128], BF, tag="psT", name="psT2", bufs=2)
            nc.tensor.transpose(psT2[:], flat[:, 128:160], id128_sb[:])
            nc.vector.tensor_copy(flatTb[:, bh * 128:(bh + 1) * 128], psT2[:])
        # fc1: h1 = relu(w1 @ flat + b1)   [512, 256]
        for fc in range(4):
            ps1 = ps_r.tile([128, 256], F32, tag="psD", name="ps1", bufs=2)
            nc.tensor.matmul(ps1[:], w1ta_sb[:, fc * 128:(fc + 1) * 128], flatTa[:],
                             start=True, stop=False)
            nc.tensor.matmul(ps1[:], w1tb_sb[:, fc * 128:(fc + 1) * 128], flatTb[:],
                             start=False, stop=True)
            nc.scalar.activation(h1q[fc][:], ps1[:], ACTF.Relu, bias=b1d_sb[fc][:],
                                 scale=1.0)
        # fc2: h2 = relu(w2 @ h1 + b2), quantized to fp8 * S_H2
        for gc in range(8):
            ps2 = ps_r.tile([128, 256], F32, tag="psD", name="ps2", bufs=2)
            for kc in range(4):
                nc.tensor.matmul(ps2[:], w2t_sb[kc][:, gc * 128:(gc + 1) * 128],
                                 h1q[kc][:], start=(kc == 0), stop=(kc == 3))
            nc.scalar.activation(h2q_sb[:, gc, :], ps2[:], ACTF.Relu,
                                 bias=b2s_sb[gc][:], scale=S_H2)

    # =================== final big layer (fp8 DoubleRow, weights resident) ====
    with tc.tile_pool(name="ps_o", bufs=4, space="PSUM") as ps_o, \
         tc.tile_pool(name="osb", bufs=4) as osbp:
        w3v = w3q_sb[:].rearrange("p (w r n j) -> p w r j n", w=NW, r=4, j=2)
        for w in range(NW):
            for bh in range(2):
                pso = ps_o.tile([128, 512], F32, tag="pso", name="pso")
                nc.vector.tensor_copy(pso[:], b3rep_sb[:, w * 512:(w + 1) * 512])
                for pr in range(4):
                    nc.tensor.matmul(pso[:],
                                     h2q_sb[:, 2 * pr:2 * pr + 2, bh * 128:(bh + 1) * 128],
                                     w3v[:, w, pr], start=False, stop=(pr == 3),
                                     perf_mode=PM.DoubleRow, skip_group_check=True)
                ot = osbp.tile([128, 512], BF, tag="ot", name="ot")
                nc.scalar.activation(ot[:], pso[:], ACTF.Sigmoid, scale=1.0 / (S_H2 * S_W3))
                nc.sync.dma_start(out_ext[bh * 128:(bh + 1) * 128,
                                          w * 512:(w + 1) * 512], ot[:])
    w3cm.__exit__(None, None, None)


_NC_CACHE = {}


def _host_prep(inputs):
    data = np.asarray(inputs["data"], np.float32)
    conv1_w = np.asarray(inputs["conv1_w"], np.float32)
    conv1_b = np.asarray(inputs["conv1_b"], np.float32)
    prim_w = np.asarray(inputs["prim_w"], np.float32)
    prim_b = np.asarray(inputs["prim_b"], np.float32)
    W_digit = np.asarray(inputs["W_digit"], np.float32)
    dec_w1 = np.asarray(inputs["dec_w1"], np.float32)
    dec_b1 = np.asarray(inputs["dec_b1"], np.float32)
    dec_w2 = np.asarray(inputs["dec_w2"], np.float32)
    dec_b2 = np.asarray(inputs["dec_b2"], np.float32)
    dec_w3 = np.asarray(inputs["dec_w3"], np.float32)
    dec_b3 = np.asarray(inputs["dec_b3"], np.float32)

    w1c = np.ascontiguousarray(conv1_w[:, 0].transpose(1, 2, 0).reshape(81, 256)).astype(BF16)
    wp2q = np.ascontiguousarray(
        prim_w.transpose(2, 3, 1, 0).reshape(20736, 256) * S_WP).astype(FP8)
    # Wfull2 [co=(cc,cl), (j(h,g,i), cd, o)]; route r = 256h + 16cc + 2cl + g
    Wv = W_digit.reshape(2, 16, 8, 2, 10, 16, 8)  # [h, cc, cl, g, cd, o, i]
    wfull = np.ascontiguousarray(Wv.transpose(1, 2, 0, 3, 6, 4, 5)).reshape(128, 5120).astype(BF16)
    w1t = np.ascontiguousarray(dec_w1.T).astype(BF16)
    w2t = np.ascontiguousarray(dec_w2.T).astype(BF16)
    w3t = np.ascontiguousarray(dec_w3.T)  # [1024, 102400]

    common = dict(
        w1c=w1c, b1s=(conv1_b * S_H).reshape(256, 1),
        bp2=prim_b.reshape(256, 1), wp2q=wp2q, wfull=wfull,
        id128=np.eye(128, dtype=np.float32).astype(BF16),
        id128f=np.eye(128, dtype=np.float32),
        w1t=w1t, b1d=dec_b1.reshape(512, 1),
        w2t=w2t, b2s=(dec_b2 * S_H2).reshape(1024, 1),
        onesrow=np.ones((1, 128), np.float32).astype(BF16),
        ones128=np.ones((128, 1), np.float32),
    )
    in_maps = []
    for c in range(NCORES):
        m = dict(common)
        sw = np.lib.stride_tricks.sliding_window_view(
            data[c * BL:(c + 1) * BL, 0], (9, 9), axis=(1, 2))
        m["pat1h"] = np.ascontiguousarray(
            sw.transpose(3, 4, 1, 2, 0).reshape(81, 4608)).astype(BF16)
        w3c = w3t[:, c * RECL:(c + 1) * RECL] * S_W3   # [1024, 12800]
        m["w3q"] = np.ascontiguousarray(
            w3c.reshape(4, 2, 128, NW, 512).transpose(2, 3, 0, 4, 1).reshape(128, 102400)
        ).astype(FP8)
        m["b3q"] = (dec_b3[c * RECL:(c + 1) * RECL] * (S_H2 * S_W3)).reshape(1, RECL).astype(BF16)
        in_maps.append(m)
    return in_maps


def kernel(**inputs):
    if "nc" not in _NC_CACHE:
        _NC_CACHE["nc"] = build_program()
    nc = _NC_CACHE["nc"]
    in_maps = _host_prep(inputs)
    res = run_bass_kernel_spmd(nc, in_maps, list(range(NCORES)))
    outs = [np.asarray(res.results[c]["out"]).astype(np.float32) for c in range(NCORES)]
    rec = np.concatenate(outs, axis=1)
    return rec.reshape(B, 256, 20, 20)


# revision 26
# speedup vs baseline: 1.6088x; 1.0073x over previous
import sys

sys.path.insert(0, "/opt/trn_rl_repo")

import numpy as np
import ml_dtypes

import concourse.bass as bass
import concourse.mybir as mybir
import concourse.tile as tile
from concourse import bacc
from concourse.bass_utils import run_bass_kernel_spmd

BF16 = ml_dtypes.bfloat16
FP8 = ml_dtypes.float8_e4m3
F32 = mybir.dt.float32
BF = mybir.dt.bfloat16
F8 = mybir.dt.float8e4
ALU = mybir.AluOpType
ACTF = mybir.ActivationFunctionType
AX = mybir.AxisListType
PM = mybir.MatmulPerfMode

NCORES = 8
B = 256
BL = B // NCORES          # 32 local batch
REC = 102400
RECL = REC // NCORES      # 12800 local output cols
NW = RECL // 512          # 25 output windows

S_H = 32.0                # conv1-activation fp8 scale
S_WP = 4096.0             # primary-caps weight fp8 scale
S_H2 = 2.0 ** 23          # h2 fp8 scale
S_W3 = 1024.0             # dec_w3 fp8 scale


def mkap(t, offset, dims):
    """Manual access pattern: dims = [[stride, count], ...] (partition dim first)."""
    return bass.AP(tensor=t.tensor if isinstance(t, bass.AP) else t, offset=offset, ap=dims)


def build_program():
    nc = bacc.Bacc(None, num_devices=NCORES)
    rg = [list(range(NCORES))]

    P = {}
    out_ext = nc.declare_dram_parameter("out", [B, RECL], BF, isOutput=True)
    P["w3q"] = nc.declare_dram_parameter("w3q", [128, 102400], F8, isOutput=False)
    P["w2t"] = nc.declare_dram_parameter("w2t", [512, 1024], BF, isOutput=False)
    P["wfull"] = nc.declare_dram_parameter("wfull", [128, 5120], BF, isOutput=False)
    P["b3q"] = nc.declare_dram_parameter("b3q", [1, RECL], BF, isOutput=False)
    P["w1t"] = nc.declare_dram_parameter("w1t", [160, 512], BF, isOutput=False)
    P["b1d"] = nc.declare_dram_parameter("b1d", [512, 1], F32, isOutput=False)
    P["b2s"] = nc.declare_dram_parameter("b2s", [1024, 1], F32, isOutput=False)
    P["id128"] = nc.declare_dram_parameter("id128", [128, 128], BF, isOutput=False)
    P["id128f"] = nc.declare_dram_parameter("id128f", [128, 128], F32, isOutput=False)
    P["onesrow"] = nc.declare_dram_parameter("onesrow", [1, 128], BF, isOutput=False)
    P["ones128"] = nc.declare_dram_parameter("ones128", [128, 1], F32, isOutput=False)
    P["wp2q"] = nc.declare_dram_parameter("wp2q", [20736, 256], F8, isOutput=False)
    P["bp2"] = nc.declare_dram_parameter("bp2", [256, 1], F32, isOutput=False)
    P["b1s"] = nc.declare_dram_parameter("b1s", [256, 1], F32, isOutput=False)
    P["w1c"] = nc.declare_dram_parameter("w1c", [81, 256], BF, isOutput=False)
    P["pat1h"] = nc.declare_dram_parameter("pat1h", [81, 4608], BF, isOutput=False)

    with tile.TileContext(nc) as tc:
        _body(nc, tc, P, out_ext, rg)
    nc.compile()
    return nc


def _body(nc, tc, P, out_ext, rg):
    es = tc.tile_pool(name="const", bufs=1)
    const = es.__enter__()
    dram_cm = tc.tile_pool(name="dram", bufs=1, space="DRAM")
    dram = dram_cm.__enter__()

    # ---------- DRAM scratch ----------
    warm_in = dram.tile([1, 16], F32, tag="warm_in", name="warm_in")
    warm_out = dram.tile([1, 16], F32, tag="warm_out", name="warm_out")
    xdram = dram.tile([2, 128, 512], BF, tag="xdram", name="xdram")
    warm2_in = dram.tile([40, 128], F32, tag="warm2_in", name="warm2_in")
    warm2_out = dram.tile([40, 128], F32, tag="warm2_out", name="warm2_out")
    ar_in = dram.tile([40, 128], F32, tag="ar_in", name="ar_in")
    ar_out = dram.tile([40, 128], F32, tag="ar_out", name="ar_out")
    c2d = dram.tile([10, 512], BF, tag="c2d", name="c2d")
    z_in = dram.tile([1, 16], F32, tag="z_in", name="z_in")
    z_out = dram.tile([1, 16], F32, tag="z_out", name="z_out")
    vin = dram.tile([BL, 160], F32, tag="vin", name="vin")
    vall = dram.tile([NCORES, BL, 160], F32, tag="vall", name="vall")

    # conv1 im2col patches: most critical load, on the gpsimd ring (less
    # contended by the input-upload traffic at kernel start)
    pat1 = const.tile([81, 4608], BF, tag="pat1", name="pat1")
    nc.gpsimd.dma_start(pat1[:], P["pat1h"][:])

    # ---------- constants to SBUF ----------
    w1c_sb = const.tile([81, 256], BF, tag="w1c", name="w1c")
    nc.gpsimd.dma_start(w1c_sb[:], P["w1c"][:])
    # warmup collectives (absorb first-collective setup cost); queued on
    # gpsimd after the two critical front loads
    zw2 = const.tile([40, 128], F32, tag="zw2", name="zw2")
    nc.vector.memset(zw2[:], 0)
    nc.sync.dma_start(warm2_in[:], zw2[:])
    nc.gpsimd.collective_compute(
        "AllReduce", ALU.add, replica_groups=rg,
        ins=[warm2_in[:].opt()], outs=[warm2_out[:].opt()])
    b1s_sb = [const.tile([128, 1], F32, tag=f"b1s{h}", name=f"b1s{h}") for h in range(2)]
    bp2_sb = [const.tile([128, 1], F32, tag=f"bp2{h}", name=f"bp2{h}") for h in range(2)]
    for h in range(2):
        nc.sync.dma_start(b1s_sb[h][:], P["b1s"][h * 128:(h + 1) * 128, :])
        nc.sync.dma_start(bp2_sb[h][:], P["bp2"][h * 128:(h + 1) * 128, :])
    wfull_sb = const.tile([128, 5120], BF, tag="wfull", name="wfull")
    id128_sb = const.tile([128, 128], BF, tag="id128", name="id128")
    nc.sync.dma_start(id128_sb[:], P["id128"][:])
    id128f_sb = const.tile([128, 128], F32, tag="id128f", name="id128f")
    nc.sync.dma_start(id128f_sb[:], P["id128f"][:])
    w1ta_sb = const.tile([128, 512], BF, tag="w1ta", name="w1ta")
    nc.sync.dma_start(w1ta_sb[:], P["w1t"][0:128, :])
    w1tb_sb = const.tile([32, 512], BF, tag="w1tb", name="w1tb")
    nc.sync.dma_start(w1tb_sb[:], P["w1t"][128:160, :])
    b1d_sb = [const.tile([128, 1], F32, tag=f"b1d{i}", name=f"b1d{i}") for i in range(4)]
    for i in range(4):
        nc.sync.dma_start(b1d_sb[i][:], P["b1d"][i * 128:(i + 1) * 128, :])
    w2t_sb = [const.tile([128, 1024], BF, tag=f"w2t{i}", name=f"w2t{i}") for i in range(4)]
    b2s_sb = [const.tile([128, 1], F32, tag=f"b2s{i}", name=f"b2s{i}") for i in range(8)]
    for i in range(8):
        nc.sync.dma_start(b2s_sb[i][:], P["b2s"][i * 128:(i + 1) * 128, :])
    onesrow_sb = const.tile([1, 128], BF, tag="onesrow", name="onesrow")
    nc.sync.dma_start(onesrow_sb[:], P["onesrow"][:])
    ones128_sb = const.tile([128, 1], F32, tag="ones128", name="ones128")
    nc.sync.dma_start(ones128_sb[:], P["ones128"][:])

    # persistent mid-size tiles
    X = [const.tile([128, 512], BF, tag=f"X{h}", name=f"X{h}") for h in range(2)]
    x2_sb = const.tile([BL, 4096], BF, tag="x2", name="x2")       # [b, (j,co)]
    wc_sb = const.tile([128, 5120], BF, tag="wc", name="wc")      # c-weighted W / prod scratch
    h2q_sb = const.tile([128, 8, 256], F8, tag="h2q", name="h2q")  # [feat%128, kc, b]

    # =================== conv1 + primary caps (fp8 DoubleRow) ===================
    with tc.tile_pool(name="front", bufs=1) as front, \
         tc.tile_pool(name="ps_f", bufs=2, space="PSUM") as ps_f:
        H = front.tile([128, 4608, 2], F8, tag="H", name="H")
        # full primary-caps weight, preloaded (k-pairs in dim1)
        wp2q_sb = front.tile([128, 162, 256], F8, tag="wp2q", name="wp2q")
        for k in range(3):
            nc.sync.dma_start(wp2q_sb[:, 54 * k:54 * (k + 1), :],
                              mkap(P["wp2q"], 54 * k * 32768,
                                   [[256, 128], [32768, 54], [1, 256]]))
        for h in range(2):
            for w in range(9):
                ps = ps_f.tile([128, 512], F32, tag="c1ps", name="c1ps")
                nc.tensor.matmul(ps[:], w1c_sb[:, h * 128:(h + 1) * 128],
                                 pat1[:, w * 512:(w + 1) * 512],
                                 start=True, stop=True)
                nc.scalar.activation(H[:, w * 512:(w + 1) * 512, h], ps[:],
                                     ACTF.Relu, bias=b1s_sb[h][:], scale=S_H)
        U = [front.tile([128, 512], F32, tag=f"U{h}", name=f"U{h}") for h in range(2)]
        psU = [ps_f.tile([128, 512], F32, tag=f"Ups{h}", name=f"Ups{h}", bufs=1) for h in range(2)]
        Hv = H[:].rearrange("p (y x b) c -> p c y x b", y=12, x=12)
        for h in range(2):
            for u in range(81):
                dy, dx = divmod(u, 9)
                rhs = Hv[:, :, dy:dy + 4, dx:dx + 4, :]
                nc.tensor.matmul(psU[h][:],
                                 wp2q_sb[:, 2 * u:2 * u + 2, h * 128:(h + 1) * 128],
                                 rhs, start=(u == 0), stop=(u == 80),
                                 perf_mode=PM.DoubleRow)
        for h in range(2):
            nc.scalar.activation(U[h][:], psU[h][:], ACTF.Identity,
                                 bias=bp2_sb[h][:], scale=1.0 / (S_H * S_WP))
        # big weight loads, delayed so they don't contend with the front stream
        nc.sync.dma_start(wfull_sb[:], P["wfull"][:])
        for i in range(4):
            nc.sync.dma_start(w2t_sb[i][:], P["w2t"][i * 128:(i + 1) * 128, :])

        # ---- squash -> x (bf16), to DRAM, reload transposed ----
        usq = front.tile([128, 512], F32, tag="usq", name="usq")
        sn = front.tile([128, 64], F32, tag="sn", name="sn")
        g = front.tile([128, 64], F32, tag="g", name="g")
        gt = front.tile([128, 64], F32, tag="gt", name="gt")
        for h in range(2):
            nc.vector.tensor_tensor(usq[:], U[h][:], U[h][:], op=ALU.mult)
            uview = usq[:].rearrange("p (g i b) -> p g b i", g=2, i=8)
            nc.vector.tensor_reduce(sn[:].rearrange("p (g b) -> p g b", g=2),
                                    uview, axis=AX.X, op=ALU.add)
            nc.scalar.activation(gt[:], sn[:], ACTF.Sqrt)
            nc.vector.tensor_scalar_add(g[:], sn[:], 1.0)
            nc.vector.reciprocal(g[:], g[:])
            nc.vector.tensor_tensor(g[:], g[:], gt[:], op=ALU.mult)
            gb = g[:].rearrange("p (g b) -> p g b", g=2).unsqueeze(2).broadcast_to(
                [128, 2, 8, BL])
            nc.vector.tensor_tensor(X[h][:].rearrange("p (g i b) -> p g i b", g=2, i=8),
                                    U[h][:].rearrange("p (g i b) -> p g i b", g=2, i=8),
                                    gb, op=ALU.mult)

    w3cm = tc.tile_pool(name="w3pool", bufs=1)
    w3pool = w3cm.__enter__()
    w3q_sb = w3pool.tile([128, 102400], F8, tag="w3q", name="w3q")
    nc.scalar.dma_start(w3q_sb[:], P["w3q"][:])
    b3rep_sb = w3pool.tile([128, RECL], BF, tag="b3rep", name="b3rep")
    nc.gpsimd.dma_start(b3rep_sb[:], mkap(P["b3q"], 0, [[0, 128], [1, RECL]]))

    # =================== routing (3 iters, matmul-factored) ===================
    with tc.tile_pool(name="route", bufs=1) as rt, \
         tc.tile_pool(name="ps_r", bufs=1, space="PSUM") as ps_r:
        s_sb = rt.tile([BL, 160], F32, tag="s_sb", name="s_sb")
        sq = rt.tile([BL, 160], F32, tag="sq", name="sq")
        num = rt.tile([BL, 160], F32, tag="num", name="num")
        dn = rt.tile([BL, 160], F32, tag="dn", name="dn")
        v_sb = rt.tile([BL, 160], F32, tag="v_sb", name="v_sb")
        vq = rt.tile([BL, 160], BF, tag="vq", name="vq")
        arin_sb = rt.tile([40, 128], F32, tag="arin_sb", name="arin_sb")
        b_accT = rt.tile([40, 128], F32, tag="b_accT", name="b_accT")
        braw = rt.tile([128, 40], F32, tag="braw", name="braw")
        csf = rt.tile([10, 512], F32, tag="csf", name="csf")
        rmax = rt.tile([10, 1], F32, tag="rmax", name="rmax")
        nbias = rt.tile([10, 1], F32, tag="nbias", name="nbias")
        esum = rt.tile([10, 1], F32, tag="esum", name="esum")
        c_sb = rt.tile([10, 512], F32, tag="c_sb", name="c_sb")
        cT = rt.tile([128, 40], F32, tag="cT", name="cT")

        def xsl(j):
            h, gg, ii = j >> 4, (j >> 3) & 1, j & 7
            return X[h][:].rearrange("p (g i b) -> p g i b", g=2, i=8)[:, gg, ii, :]

        for it in range(3):
            # ---- s[b,cd,o] = sum_{r,i} c.W.x  via 32 accumulated matmuls ----
            psS = ps_r.tile([BL, 160], F32, tag="psS", name="psS")
            if it == 0:
                for j in range(32):
                    nc.tensor.matmul(psS[:], xsl(j),
                                     wfull_sb[:, j * 160:(j + 1) * 160],
                                     start=(j == 0), stop=(j == 31))
                nc.vector.tensor_scalar(s_sb[:], psS[:], 1.0 / 512.0, None, op0=ALU.mult)
            else:
                cvv = cT[:].rearrange("p (hg c) -> p hg c", hg=4).unsqueeze(2)\
                    .unsqueeze(4)
                wfv = wfull_sb[:].rearrange("p (hg i c o) -> p hg i c o", hg=4, i=8, c=10)
                wcv = wc_sb[:].rearrange("p (hg i c o) -> p hg i c o", hg=4, i=8, c=10)
                for seg in range(4):
                    nc.vector.tensor_tensor(
                        wcv[:, seg], wfv[:, seg],
                        cvv[:, seg].broadcast_to([128, 8, 10, 16]), op=ALU.mult)
                    for j in range(8 * seg, 8 * (seg + 1)):
                        nc.tensor.matmul(psS[:], xsl(j),
                                         wc_sb[:, j * 160:(j + 1) * 160],
                                         start=(j == 0), stop=(j == 31))
                nc.vector.tensor_copy(s_sb[:], psS[:])
            # ---- elementwise squash: v = sq*s/((1+sq)*sqrt(sq)) ----
            nc.vector.tensor_tensor(sq[:], s_sb[:], s_sb[:], op=ALU.mult)
            nc.vector.tensor_tensor(num[:], sq[:], s_sb[:], op=ALU.mult)
            nc.vector.tensor_scalar_add(dn[:], sq[:], 1.0)
            nc.scalar.activation(sq[:], sq[:], ACTF.Sqrt)
            nc.vector.tensor_tensor(dn[:], dn[:], sq[:], op=ALU.mult)
            nc.vector.reciprocal(dn[:], dn[:])
            nc.vector.tensor_tensor(v_sb[:], num[:], dn[:], op=ALU.mult)

            if it == 2:
                break
            if it == 0:
                # x2[b, (j,co)] via 32 tensor transposes of X slices
                for j in range(32):
                    psT = ps_r.tile([32, 128], BF, tag="psT", name="psT", bufs=2)
                    nc.tensor.transpose(psT[:], xsl(j), id128_sb[:])
                    nc.vector.tensor_copy(x2_sb[:, j * 128:(j + 1) * 128], psT[:])
            # ---- G[(r,i),(cd,o)] = sum_b x v; prod = G.W fused from psum ----
            nc.vector.tensor_copy(vq[:], v_sb[:])
            for jp in range(16):
                psG = ps_r.tile([128, 320], F32, tag="psG", name="psG", bufs=2)
                for k in range(2):
                    j = 2 * jp + k
                    nc.tensor.matmul(psG[:, 160 * k:160 * (k + 1)],
                                     x2_sb[:, j * 128:(j + 1) * 128], vq[:],
                                     start=True, stop=True)
                nc.vector.tensor_tensor(wc_sb[:, jp * 320:(jp + 1) * 320], psG[:],
                                        wfull_sb[:, jp * 320:(jp + 1) * 320], op=ALU.mult)
            # ---- a_mean[r,cd] = sum_{i,o} prod : one strided XY reduce ----
            prv = wc_sb[:].rearrange("p (h g i c o) -> p h g c i o", h=2, g=2, i=8, c=10, o=16)
            nc.vector.tensor_reduce(
                braw[:].rearrange("p (c h g) -> p h g c", c=10, h=2),
                prv, axis=AX.XY, op=ALU.add)
            # transpose to [(cd,h,g), co] and fold previous b-state (/256 batch,
            # /8 cores so the AllReduce sum is the new b directly)
            psB = ps_r.tile([40, 128], F32, tag="psA", name="psB")
            nc.tensor.transpose(psB[:], braw[:], id128f_sb[:])
            if it == 0:
                nc.vector.tensor_scalar(arin_sb[:], psB[:], 1.0 / 256.0, None, op0=ALU.mult)
            else:
                nc.vector.scalar_tensor_tensor(arin_sb[:], psB[:], 1.0 / 256.0,
                                               b_accT[:], op0=ALU.mult, op1=ALU.add)
            nc.sync.dma_start(ar_in[:], arin_sb[:])
            nc.gpsimd.collective_compute(
                "AllReduce", ALU.add, replica_groups=rg,
                ins=[ar_in[:].opt()], outs=[ar_out[:].opt()])
            # b_accT := b_new/8  (next round each core contributes b/8 so the
            # 8-way sum reconstructs b)
            nc.sync.dma_start(csf[:], mkap(ar_out[:], 0,
                                           [[512, 10], [256, 2], [128, 2], [1, 128]]))
            nc.gpsimd.dma_start(b_accT[:], ar_out[:])
            nc.vector.tensor_scalar(b_accT[:], b_accT[:], 0.125, None, op0=ALU.mult)
            # ---- softmax over routes (free dim) ----
            nc.vector.tensor_reduce(rmax[:], csf[:], axis=AX.X, op=ALU.max)
            nc.scalar.mul(nbias[:], rmax[:], -1.0)
            nc.scalar.activation(c_sb[:], csf[:], ACTF.Exp, bias=nbias[:], scale=1.0)
            nc.vector.tensor_reduce(esum[:], c_sb[:], axis=AX.X, op=ALU.add)
            nc.vector.reciprocal(esum[:], esum[:])
            nc.vector.tensor_scalar_mul(c_sb[:], c_sb[:], esum[:])
            # ---- cT[co, (h,g,cd)] via 4 on-chip transposes ----
            for hg in range(4):
                psC = ps_r.tile([128, 16], F32, tag="psT", name="psC", bufs=2)
                nc.tensor.transpose(psC[:, :10], c_sb[:, hg * 128:(hg + 1) * 128],
                                    id128f_sb[:10, :10])
                nc.vector.tensor_copy(cT[:, hg * 10:(hg + 1) * 10], psC[:, :10])

        # =================== tail: AllGather v, full-batch decoder ===========
        nc.sync.dma_start(vin[:], v_sb[:])
        nc.gpsimd.collective_compute(
            "AllGather", ALU.bypass, replica_groups=rg,
            ins=[vin[:].opt()], outs=[vall[:].opt()])

        vfull = [rt.tile([128, 160], F32, tag=f"vf{bh}", name=f"vf{bh}") for bh in range(2)]
        ecl = [rt.tile([128, 10], F32, tag=f"ecl{bh}", name=f"ecl{bh}") for bh in range(2)]
        sqf = rt.tile([128, 160], F32, tag="sqf", name="sqf")
        cl = rt.tile([128, 10], F32, tag="cl", name="cl")
        psZ = ps_r.tile([1, 16], F32, tag="psA", name="psZ", bufs=1)
        for bh in range(2):
            nc.sync.dma_start(vfull[bh][:],
                              mkap(vall[:], bh * 128 * 160, [[160, 128], [1, 160]]))
            nc.vector.tensor_tensor(sqf[:], vfull[bh][:], vfull[bh][:], op=ALU.mult)
            nc.vector.tensor_reduce(cl[:], sqf[:].rearrange("p (c o) -> p c o", c=10),
                                    axis=AX.X, op=ALU.add)
            nc.scalar.activation(cl[:], cl[:], ACTF.Sqrt)
            nc.scalar.activation(ecl[bh][:], cl[:], ACTF.Exp)
            nc.tensor.matmul(psZ[:, :10], ones128_sb[:], ecl[bh][:],
                             start=(bh == 0), stop=(bh == 1))
        zrow = rt.tile([1, 16], F32, tag="zrow", name="zrow")
        nc.vector.memset(zrow[:], 0)
        nc.vector.tensor_copy(zrow[:, :10], psZ[:, :10])
        nc.vector.reciprocal(zrow[:, :10], zrow[:, :10])
        zrowb = rt.tile([1, 16], BF, tag="zrowb", name="zrowb")
        nc.vector.tensor_copy(zrowb[:], zrow[:])
        psZb = ps_r.tile([128, 16], F32, tag="psA", name="psZb")
        nc.tensor.matmul(psZb[:, :10], onesrow_sb[:], zrowb[:, :10],
                         start=True, stop=True)
        zfull = psZb

        tpr = rt.tile([128, 10], F32, tag="tpr", name="tpr")
        tmax = rt.tile([128, 1], F32, tag="tmax", name="tmax")
        mask = rt.tile([128, 10], F32, tag="mask", name="mask")
        flat = rt.tile([128, 160], BF, tag="flat", name="flat")
        flatTa = rt.tile([128, 256], BF, tag="flatTa", name="flatTa")
        flatTb = rt.tile([32, 256], BF, tag="flatTb", name="flatTb")
        h1q = [rt.tile([128, 256], BF, tag=f"h1q{i}", name=f"h1q{i}") for i in range(4)]
        for bh in range(2):
            nc.vector.tensor_tensor(tpr[:], ecl[bh][:], zfull[:, :10], op=ALU.mult)
            nc.vector.tensor_reduce(tmax[:], tpr[:], axis=AX.X, op=ALU.max)
            nc.vector.tensor_scalar(mask[:], tpr[:], tmax[:], None, op0=ALU.is_equal)
            mb = mask[:].unsqueeze(2).broadcast_to([128, 10, 16])
            nc.vector.tensor_tensor(flat[:].rearrange("p (c o) -> p c o", c=10),
                                    vfull[bh][:].rearrange("p (c o) -> p c o", c=10),
                                    mb, op=ALU.mult)
            psT1 = ps_r.tile([128, 128], BF, tag="psT", name="psT1", bufs=2)
            nc.tensor.transpose(psT1[:], flat[:, 0:128], id128_sb[:])
            nc.vector.tensor_copy(flatTa[:, bh * 128:(bh + 1) * 128], psT1[:])
            psT2 = ps_r.tile([32, 128], BF, tag="psT", name="psT2", bufs=2)
            nc.tensor.transpose(psT2[:], flat[:, 128:160], id128_sb[:])
            nc.vector.tensor_copy(flatTb[:, bh * 128:(bh + 1) * 128], psT2[:])
        # fc1: h1 = relu(w1 @ flat + b1)   [512, 256]
        for fc in range(4):
            ps1 = ps_r.tile([128, 256], F32, tag="psD", name="ps1", bufs=2)
            nc.tensor.matmul(ps1[:], w1ta_sb[:, fc * 128:(fc + 1) * 128], flatTa[:],
                             start=True, stop=False)
            nc.tensor.matmul(ps1[:], w1tb_sb[:, fc * 128:(fc + 1) * 128], flatTb[:],
                             start=False, stop=True)
            nc.scalar.activation(h1q[fc][:], ps1[:], ACTF.Relu, bias=b1d_sb[fc][:],
                                 scale=1.0)
        # fc2: h2 = relu(w2 @ h1 + b2), quantized to fp8 * S_H2
        for gc in range(8):
            ps2 = ps_r.tile([128, 256], F32, tag="psD", name="ps2", bufs=2)
            for kc in range(4):
                nc.tensor.matmul(ps2[:], w2t_sb[kc][:, gc * 128:(gc + 1) * 128],
                                 h1q[kc][:], start=(kc == 0), stop=(kc == 3))
            nc.scalar.activation(h2q_sb[:, gc, :], ps2[:], ACTF.Relu,
                                 bias=b2s_sb[gc][:], scale=S_H2)

    # =================== final big layer (fp8 DoubleRow, weights resident) ====
    with tc.tile_pool(name="ps_o", bufs=4, space="PSUM") as ps_o, \
         tc.tile_pool(name="osb", bufs=4) as osbp:
        w3v = w3q_sb[:].rearrange("p (w r n j) -> p w r j n", w=NW, r=4, j=2)
        for w in range(NW):
            for bh in range(2):
                pso = ps_o.tile([128, 512], F32, tag="pso", name="pso")
                nc.vector.tensor_copy(pso[:], b3rep_sb[:, w * 512:(w + 1) * 512])
                for pr in range(4):
                    nc.tensor.matmul(pso[:],
                                     h2q_sb[:, 2 * pr:2 * pr + 2, bh * 128:(bh + 1) * 128],
                                     w3v[:, w, pr], start=False, stop=(pr == 3),
                                     perf_mode=PM.DoubleRow, skip_group_check=True)
                ot = osbp.tile([128, 512], BF, tag="ot", name="ot")
                nc.scalar.activation(ot[:], pso[:], ACTF.Sigmoid, scale=1.0 / (S_H2 * S_W3))
                nc.sync.dma_start(out_ext[bh * 128:(bh + 1) * 128,
                                          w * 512:(w + 1) * 512], ot[:])
    w3cm.__exit__(None, None, None)


_NC_CACHE = {}


def _host_prep(inputs):
    data = np.asarray(inputs["data"], np.float32)
    conv1_w = np.asarray(inputs["conv1_w"], np.float32)
    conv1_b = np.asarray(inputs["conv1_b"], np.float32)
    prim_w = np.asarray(inputs["prim_w"], np.float32)
    prim_b = np.asarray(inputs["prim_b"], np.float32)
    W_digit = np.asarray(inputs["W_digit"], np.float32)
    dec_w1 = np.asarray(inputs["dec_w1"], np.float32)
    dec_b1 = np.asarray(inputs["dec_b1"], np.float32)
    dec_w2 = np.asarray(inputs["dec_w2"], np.float32)
    dec_b2 = np.asarray(inputs["dec_b2"], np.float32)
    dec_w3 = np.asarray(inputs["dec_w3"], np.float32)
    dec_b3 = np.asarray(inputs["dec_b3"], np.float32)

    w1c = np.ascontiguousarray(conv1_w[:, 0].transpose(1, 2, 0).reshape(81, 256)).astype(BF16)
    wp2q = np.ascontiguousarray(
        prim_w.transpose(2, 3, 1, 0).reshape(20736, 256) * S_WP).astype(FP8)
    # Wfull2 [co=(cc,cl), (j(h,g,i), cd, o)]; route r = 256h + 16cc + 2cl + g
    Wv = W_digit.reshape(2, 16, 8, 2, 10, 16, 8)  # [h, cc, cl, g, cd, o, i]
    wfull = np.ascontiguousarray(Wv.transpose(1, 2, 0, 3, 6, 4, 5)).reshape(128, 5120).astype(BF16)
    w1t = np.ascontiguousarray(dec_w1.T).astype(BF16)
    w2t = np.ascontiguousarray(dec_w2.T).astype(BF16)
    w3t = np.ascontiguousarray(dec_w3.T)  # [1024, 102400]

    common = dict(
        w1c=w1c, b1s=(conv1_b * S_H).reshape(256, 1),
        bp2=prim_b.reshape(256, 1), wp2q=wp2q, wfull=wfull,
        id128=np.eye(128, dtype=np.float32).astype(BF16),
        id128f=np.eye(128, dtype=np.float32),
        w1t=w1t, b1d=dec_b1.reshape(512, 1),
        w2t=w2t, b2s=(dec_b2 * S_H2).reshape(1024, 1),
        onesrow=np.ones((1, 128), np.float32).astype(BF16),
        ones128=np.ones((128, 1), np.float32),
    )
    in_maps = []
    for c in range(NCORES):
        m = dict(common)
        sw = np.lib.stride_tricks.sliding_window_view(
            data[c * BL:(c + 1) * BL, 0], (9, 9), axis=(1, 2))
        m["pat1h"] = np.ascontiguousarray(
            sw.transpose(3, 4, 1, 2, 0).reshape(81, 4608)).astype(BF16)
        w3c = w3t[:, c * RECL:(c + 1) * RECL] * S_W3   # [1024, 12800]
        m["w3q"] = np.ascontiguousarray(
            w3c.reshape(4, 2, 128, NW, 512).transpose(2, 3, 0, 4, 1).reshape(128, 102400)
        ).astype(FP8)
        m["b3q"] = (dec_b3[c * RECL:(c + 1) * RECL] * (S_H2 * S_W3)).reshape(1, RECL).astype(BF16)
        in_maps.append(m)
    return in_maps


def kernel(**inputs):
    if "nc" not in _NC_CACHE:
        _NC_CACHE["nc"] = build_program()
    nc = _NC_CACHE["nc"]
    in_maps = _host_prep(inputs)
    res = run_bass_kernel_spmd(nc, in_maps, list(range(NCORES)))
    outs = [np.asarray(res.results[c]["out"]).astype(np.float32) for c in range(NCORES)]
    rec = np.concatenate(outs, axis=1)
    return rec.reshape(B, 256, 20, 20)
